# revision 10
# baseline (speedup 1.0000x reference)
"""Trainium2 Bass kernel for nn_DechunkingLayer.

Full-input contract: kernel(z, p, b, original_len) with
  z [8, 1024, 1024] f32, p [8, 4096] f32, b [8, 4096] i32  ->  [8, 4096, 1024] f32

Sharding: data-parallel over batch — core i processes row i (cumsum / gather /
roll are independent per batch row).

v2 design (fp16 staging, fused shift+blend matmul):
  host:   idx = clip(cumsum(b)-b, 0, Lz-1)  (pure input marshalling)
          z16 = z.astype(f16); output returned as f16 -> f32 host-side.
          Halves device HBM traffic (16.5MB vs 33.7MB) and tunnel bytes.
  device, per 128-row t-tile g:
          up = z16[idx[t]]                 # gpsimd indirect gather, f16 rows
          ps = C_g @ up (+ halo)           # ONE PE matmul: C_g has p on the
                                           # diag and q=1-p on the subdiag, so
                                           # it does roll+blend in one pass;
                                           # the cross-tile halo row is a 2nd
                                           # tiny accumulate-matmul with
                                           # q[128g] at lhsT row 127.
          out[g] = ps (f16)                # ACT+DVE copy PSUM->SBUF, DMA out
  C_g^T is built on-chip: Pb = ones^T @ p_row (PE broadcast), then
  C^T = Ssub + (Id - Ssub) * Pb (2 DVE ops on [128,128] f16).
"""

import numpy as np

import concourse.bass as bass
import concourse.bacc as bacc
import concourse.tile as tile
from concourse import mybir
from concourse.bass_utils import run_bass_kernel_spmd

P = 128       # partitions / t-tile height
G = 32        # t-tiles = T // P
T = 4096
LZ = 1024
D = 1024
N_CORES = 8

F32 = mybir.dt.float32
F16 = mybir.dt.float16
I32 = mybir.dt.int32
ALU = mybir.AluOpType
ACTF = mybir.ActivationFunctionType

GCOLS = 1   # t-tiles gathered per indirect_dma_start (>1 is broken: device abort)
GATHER_MODE = "indirect"   # "indirect" | "dma_gather"
CH = 4      # t-tiles per dma_gather call


def _const_inputs_v2() -> dict[str, np.ndarray]:
    return {
        "ssub": np.eye(P, k=1, dtype=np.float16),   # lhsT[k,t]=1 iff k==t-1
        "dpm": (np.eye(P, dtype=np.float32)
                - np.eye(P, k=1, dtype=np.float32)).astype(np.float16),
        "ones1": np.ones((1, P), dtype=np.float16),
    }


def build_nc_v2(gcols: int | None = None, gather_mode: str | None = None) -> bacc.Bacc:
    if gcols is None:
        gcols = GCOLS
    if gather_mode is None:
        gather_mode = GATHER_MODE
    assert G % gcols == 0
    nc = bacc.Bacc("TRN2", target_bir_lowering=False, debug=False)

    z_d = nc.dram_tensor("z16", [LZ, D], F16, kind="ExternalInput")
    p_d = nc.dram_tensor("p16", [1, T], F16, kind="ExternalInput")
    idx_d = nc.dram_tensor("idxc", [P, G], I32, kind="ExternalInput")
    if gather_mode == "dma_gather":
        w16_d = nc.dram_tensor("w16i", [P, T // 16], mybir.dt.int16,
                               kind="ExternalInput")
    hoff_d = nc.dram_tensor("hoff", [G, 1], I32, kind="ExternalInput")
    hoffp_d = nc.dram_tensor("hoffp", [G, 1], I32, kind="ExternalInput")
    qh_d = nc.dram_tensor("qhc", [G, 1], F32, kind="ExternalInput")
    ph_d = nc.dram_tensor("phc", [G, 1], F32, kind="ExternalInput")
    ssub_d = nc.dram_tensor("ssub", [P, P], F16, kind="ExternalInput")
    dpm_d = nc.dram_tensor("dpm", [P, P], F16, kind="ExternalInput")
    ones1_d = nc.dram_tensor("ones1", [1, P], F16, kind="ExternalInput")
    out_d = nc.dram_tensor("out", [T, D], F16, kind="ExternalOutput")

    with tile.TileContext(nc) as tc:
        with (
            tc.tile_pool(name="consts", bufs=1) as cpool,
            tc.tile_pool(name="up", bufs=8) as upool,
            tc.tile_pool(name="ct", bufs=2) as ctpool,
            tc.tile_pool(name="cttmp", bufs=2) as tmppool,
            tc.tile_pool(name="outp", bufs=4) as opool,
            tc.tile_pool(name="pbps", bufs=1, space="PSUM") as pbpsum,
            tc.tile_pool(name="psum", bufs=3, space="PSUM") as ppool,
        ):
            # ---- constants / small inputs (idxc first: gathers need it) ----
            idxc = cpool.tile([P, G], I32)
            nc.sync.dma_start(idxc[:], idx_d[:, :])
            if gather_mode == "dma_gather":
                w16 = cpool.tile([P, T // 16], mybir.dt.int16)
                nc.sync.dma_start(w16[:], w16_d[:, :])
            ones1 = cpool.tile([1, P], F16)
            nc.sync.dma_start(ones1[:], ones1_d[:, :])
            p16 = cpool.tile([1, T], F16)
            nc.sync.dma_start(p16[:], p_d[:, :])
            ssub = cpool.tile([P, P], F16)
            nc.scalar.dma_start(ssub[:], ssub_d[:, :])
            dpm = cpool.tile([P, P], F16)
            nc.scalar.dma_start(dpm[:], dpm_d[:, :])
            hoff = cpool.tile([G, 1], I32)
            nc.scalar.dma_start(hoff[:], hoff_d[:, :])
            hoffp = cpool.tile([G, 1], I32)
            nc.scalar.dma_start(hoffp[:], hoffp_d[:, :])
            qh = cpool.tile([G, 1], F32)
            nc.scalar.dma_start(qh[:], qh_d[:, :])
            ph = cpool.tile([G, 1], F32)
            nc.scalar.dma_start(ph[:], ph_d[:, :])
            out_head_rows = out_d[:, :].rearrange("(g x) d -> g x d", x=P)[:, 0, :]

            # ---- broadcast p to all partitions once: pball[k, t] = p[t] ----
            pball = cpool.tile([P, T], F16)
            for j in range(T // 512):
                pb_ps = pbpsum.tile([P, 512], F32)
                nc.tensor.matmul(
                    pb_ps[:], lhsT=ones1[:], rhs=p16[0:1, j * 512 : (j + 1) * 512],
                    start=True, stop=True,
                )
                nc.scalar.activation(
                    pball[:, j * 512 : (j + 1) * 512], pb_ps[:], func=ACTF.Copy
                )

            chunk = None
            for g in range(G):
                # -- gather up[t] = z16[idx[t]] --
                if gather_mode == "dma_gather":
                    j = g % CH
                    if j == 0:
                        c = g // CH
                        ncols = CH * P // 16
                        chunk = upool.tile([P, CH, D], F16)
                        nc.gpsimd.dma_gather(
                            out_ap=chunk[:],
                            in_ap=z_d[:, :],
                            idxs_ap=w16[:, c * ncols : (c + 1) * ncols],
                            num_idxs=CH * P,
                            num_idxs_reg=CH * P,
                            elem_size=D,
                        )
                    up = chunk[:, j, :]
                elif gcols > 1:
                    j = g % gcols
                    if j == 0:
                        c = g // gcols
                        chunk = upool.tile([P, gcols, D], F16)
                        nc.gpsimd.indirect_dma_start(
                            out=chunk[:],
                            out_offset=None,
                            in_=z_d[:, :],
                            in_offset=bass.IndirectOffsetOnAxis(
                                ap=idxc[:, c * gcols : (c + 1) * gcols], axis=0
                            ),
                        )
                    up = chunk[:, j, :]
                else:
                    up_t = upool.tile([P, D], F16)
                    up = up_t[:]
                    nc.gpsimd.indirect_dma_start(
                        out=up,
                        out_offset=None,
                        in_=z_d[:, :],
                        in_offset=bass.IndirectOffsetOnAxis(
                            ap=idxc[:, g : g + 1], axis=0
                        ),
                    )

                if g == 1:
                    # head rows: out[128g] = p[128g]*z16[idx[128g]]
                    #                      + q[128g]*z16[idx[128g-1]]
                    # (partition-aligned on 32 partitions; written by an
                    # independent strided DMA - tile DMAs skip row 0)
                    haloq = cpool.tile([G, D], F16)
                    nc.gpsimd.indirect_dma_start(
                        out=haloq[:],
                        out_offset=None,
                        in_=z_d[:, :],
                        in_offset=bass.IndirectOffsetOnAxis(ap=hoff[:, 0:1], axis=0),
                    )
                    halop = cpool.tile([G, D], F16)
                    nc.gpsimd.indirect_dma_start(
                        out=halop[:],
                        out_offset=None,
                        in_=z_d[:, :],
                        in_offset=bass.IndirectOffsetOnAxis(ap=hoffp[:, 0:1], axis=0),
                    )
                    h1 = cpool.tile([G, D], F16)
                    nc.vector.tensor_scalar(
                        h1[:], halop[:], ph[:, 0:1], None, op0=ALU.mult
                    )
                    row0c = cpool.tile([G, D], F16)
                    nc.vector.scalar_tensor_tensor(
                        row0c[:], in0=haloq[:], scalar=qh[:, 0:1], in1=h1[:],
                        op0=ALU.mult, op1=ALU.add,
                    )
                    nc.sync.dma_start(out_head_rows, row0c[:])

                # -- build C_g^T = Ssub + (Id - Ssub) * broadcast(p_g) --
                tmp = tmppool.tile([P, P], F16)
                nc.vector.tensor_tensor(
                    tmp[:], dpm[:], pball[:, g * P : (g + 1) * P], ALU.mult
                )
                ct = ctpool.tile([P, P], F16)
                nc.vector.tensor_tensor(ct[:], tmp[:], ssub[:], ALU.add)

                # -- fused roll+blend matmul --
                ps = ppool.tile([P, D], F32)
                for h in range(0, D, 512):
                    nc.tensor.matmul(
                        ps[:, h : h + 512], lhsT=ct[:], rhs=up[:, h : h + 512],
                        start=True, stop=True,
                    )

                # -- PSUM -> SBUF f16, split across ACT and DVE --
                ot = opool.tile([P, D], F16)
                nc.scalar.activation(ot[:, 0:512], ps[:, 0:512], func=ACTF.Copy)
                nc.vector.tensor_copy(ot[:, 512:1024], ps[:, 512:1024])

                nc.sync.dma_start(
                    out_d[g * P + 1 : (g + 1) * P, :], ot[1:P, :]
                )


    nc.compile()
    return nc


_NC_CACHE: dict[str, bacc.Bacc] = {}


def get_nc_v2(gcols: int | None = None, gather_mode: str | None = None) -> bacc.Bacc:
    if gcols is None:
        gcols = GCOLS
    if gather_mode is None:
        gather_mode = GATHER_MODE
    key = f"v2:{gcols}:{gather_mode}"
    if key not in _NC_CACHE:
        _NC_CACHE[key] = build_nc_v2(gcols, gather_mode)
    return _NC_CACHE[key]


def make_in_maps_v2(z: np.ndarray, p: np.ndarray, b: np.ndarray) -> list[dict]:
    consts = _const_inputs_v2()
    maps = []
    for i in range(N_CORES):
        bi = b[i].astype(np.int64)
        idx = np.clip(np.cumsum(bi) - bi, 0, LZ - 1).astype(np.int32)
        idx_cm = np.ascontiguousarray(idx.reshape(G, P).T)  # [P, G]
        p16 = p[i].astype(np.float16).reshape(1, T).copy()
        # head rows: out[128g] = p[128g]*z16[idx[128g]] + q[128g]*z16[idx[128g-1]]
        # (for g=0: p=1, q=0 -> out[0] = z16[idx[0]])
        hoff = np.zeros((G, 1), dtype=np.int32)
        hoff[1:, 0] = idx[P - 1 :: P][: G - 1]
        hoffp = np.ascontiguousarray(idx[::P].reshape(G, 1))
        qh = np.zeros((G, 1), dtype=np.float32)
        qh[1:, 0] = (1.0 - p[i, P::P].astype(np.float64)).astype(np.float32)
        ph = p[i, ::P].astype(np.float32).reshape(G, 1).copy()
        ph[0, 0] = 1.0
        p16[0, 0] = 1.0
        m = {
            "z16": z[i].astype(np.float16),
            "p16": p16,
            "idxc": idx_cm,
            "hoff": hoff,
            "hoffp": hoffp,
            "qhc": qh,
            "phc": ph,
        }
        if GATHER_MODE == "dma_gather":
            # wrap-16 int16 layout: W[j, c] = idx[16c + j], replicated to 128
            w16 = np.ascontiguousarray(
                np.tile(idx.reshape(T // 16, 16).T.astype(np.int16), (P // 16, 1))
            )
            m["w16i"] = w16
        m.update(consts)
        maps.append(m)
    return maps


def run(z, p, b, **spmd_kwargs):
    nc = get_nc_v2()
    in_maps = make_in_maps_v2(z, p, b)
    res = run_bass_kernel_spmd(nc, in_maps, core_ids=list(range(N_CORES)), **spmd_kwargs)
    out = np.stack([res.results[i]["out"] for i in range(N_CORES)], axis=0)
    return out, res


def kernel(z, p, b, original_len=None, **_ignored) -> np.ndarray:
    z = np.asarray(z)
    p = np.asarray(p)
    b = np.asarray(b)
    assert z.shape == (N_CORES, LZ, D), z.shape
    assert p.shape == (N_CORES, T), p.shape
    assert b.shape == (N_CORES, T), b.shape
    out, _ = run(z, p, b)
    return out.astype(np.float32, copy=False)


# revision 12
# speedup vs baseline: 3.2608x; 3.2608x over previous
"""Trainium2 Bass kernel for nn_DechunkingLayer.

Full-input contract: kernel(z, p, b, original_len) with
  z [8, 1024, 1024] f32, p [8, 4096] f32, b [8, 4096] i32  ->  [8, 4096, 1024] f32

Sharding: data-parallel over batch — core i processes row i (cumsum / gather /
roll are independent per batch row).

v2 design (fp16 staging, fused shift+blend matmul):
  host:   idx = clip(cumsum(b)-b, 0, Lz-1)  (pure input marshalling)
          z16 = z.astype(f16); output returned as f16 -> f32 host-side.
          Halves device HBM traffic (16.5MB vs 33.7MB) and tunnel bytes.
  device, per 128-row t-tile g:
          up = z16[idx[t]]                 # gpsimd indirect gather, f16 rows
          ps = C_g @ up (+ halo)           # ONE PE matmul: C_g has p on the
                                           # diag and q=1-p on the subdiag, so
                                           # it does roll+blend in one pass;
                                           # the cross-tile halo row is a 2nd
                                           # tiny accumulate-matmul with
                                           # q[128g] at lhsT row 127.
          out[g] = ps (f16)                # ACT+DVE copy PSUM->SBUF, DMA out
  C_g^T is built on-chip: Pb = ones^T @ p_row (PE broadcast), then
  C^T = Ssub + (Id - Ssub) * Pb (2 DVE ops on [128,128] f16).
"""

import numpy as np

import concourse.bass as bass
import concourse.bacc as bacc
import concourse.tile as tile
from concourse import mybir
from concourse.bass_utils import run_bass_kernel_spmd

P = 128       # partitions / t-tile height
G = 32        # t-tiles = T // P
T = 4096
LZ = 1024
D = 1024
N_CORES = 8

F32 = mybir.dt.float32
F16 = mybir.dt.float16
I32 = mybir.dt.int32
ALU = mybir.AluOpType
ACTF = mybir.ActivationFunctionType

GCOLS = 1   # t-tiles gathered per indirect_dma_start (>1 is broken: device abort)
GATHER_MODE = "indirect"   # "indirect" | "dma_gather"
CH = 4      # t-tiles per dma_gather call


def _const_inputs_v2() -> dict[str, np.ndarray]:
    return {
        "ssub": np.eye(P, k=1, dtype=np.float16),   # lhsT[k,t]=1 iff k==t-1
        "dpm": (np.eye(P, dtype=np.float32)
                - np.eye(P, k=1, dtype=np.float32)).astype(np.float16),
        "ones1": np.ones((1, P), dtype=np.float16),
    }


def build_nc_v2(gcols: int | None = None, gather_mode: str | None = None) -> bacc.Bacc:
    if gcols is None:
        gcols = GCOLS
    if gather_mode is None:
        gather_mode = GATHER_MODE
    assert G % gcols == 0
    nc = bacc.Bacc("TRN2", target_bir_lowering=False, debug=False)

    z_d = nc.dram_tensor("z16", [LZ, D], F16, kind="ExternalInput")
    p_d = nc.dram_tensor("p16", [1, T], F16, kind="ExternalInput")
    idx_d = nc.dram_tensor("idxc", [P, G], I32, kind="ExternalInput")
    if gather_mode == "dma_gather":
        w16_d = nc.dram_tensor("w16i", [P, T // 16], mybir.dt.int16,
                               kind="ExternalInput")
    hoff_d = nc.dram_tensor("hoff", [G, 1], I32, kind="ExternalInput")
    hoffp_d = nc.dram_tensor("hoffp", [G, 1], I32, kind="ExternalInput")
    qh_d = nc.dram_tensor("qhc", [G, 1], F32, kind="ExternalInput")
    ph_d = nc.dram_tensor("phc", [G, 1], F32, kind="ExternalInput")
    ssub_d = nc.dram_tensor("ssub", [P, P], F16, kind="ExternalInput")
    dpm_d = nc.dram_tensor("dpm", [P, P], F16, kind="ExternalInput")
    ones1_d = nc.dram_tensor("ones1", [1, P], F16, kind="ExternalInput")
    out_d = nc.dram_tensor("out", [T, D], F16, kind="ExternalOutput")

    with tile.TileContext(nc) as tc:
        with (
            tc.tile_pool(name="consts", bufs=1) as cpool,
            tc.tile_pool(name="up", bufs=8) as upool,
            tc.tile_pool(name="ct", bufs=2) as ctpool,
            tc.tile_pool(name="cttmp", bufs=2) as tmppool,
            tc.tile_pool(name="outp", bufs=4) as opool,
            tc.tile_pool(name="pbps", bufs=1, space="PSUM") as pbpsum,
            tc.tile_pool(name="psum", bufs=3, space="PSUM") as ppool,
        ):
            # ---- constants / small inputs (idxc first: gathers need it) ----
            idxc = cpool.tile([P, G], I32)
            nc.sync.dma_start(idxc[:], idx_d[:, :])
            if gather_mode == "dma_gather":
                w16 = cpool.tile([P, T // 16], mybir.dt.int16)
                nc.sync.dma_start(w16[:], w16_d[:, :])
            ones1 = cpool.tile([1, P], F16)
            nc.sync.dma_start(ones1[:], ones1_d[:, :])
            p16 = cpool.tile([1, T], F16)
            nc.sync.dma_start(p16[:], p_d[:, :])
            ssub = cpool.tile([P, P], F16)
            nc.scalar.dma_start(ssub[:], ssub_d[:, :])
            dpm = cpool.tile([P, P], F16)
            nc.scalar.dma_start(dpm[:], dpm_d[:, :])
            hoff = cpool.tile([G, 1], I32)
            nc.scalar.dma_start(hoff[:], hoff_d[:, :])
            hoffp = cpool.tile([G, 1], I32)
            nc.scalar.dma_start(hoffp[:], hoffp_d[:, :])
            qh = cpool.tile([G, 1], F32)
            nc.scalar.dma_start(qh[:], qh_d[:, :])
            ph = cpool.tile([G, 1], F32)
            nc.scalar.dma_start(ph[:], ph_d[:, :])
            out_head_rows = out_d[:, :].rearrange("(g x) d -> g x d", x=P)[:, 0, :]

            # ---- broadcast p to all partitions once: pball[k, t] = p[t] ----
            pball = cpool.tile([P, T], F16)
            for j in range(T // 512):
                pb_ps = pbpsum.tile([P, 512], F32)
                nc.tensor.matmul(
                    pb_ps[:], lhsT=ones1[:], rhs=p16[0:1, j * 512 : (j + 1) * 512],
                    start=True, stop=True,
                )
                nc.scalar.activation(
                    pball[:, j * 512 : (j + 1) * 512], pb_ps[:], func=ACTF.Copy
                )

            # head rows: row0c[g] = p[128g]*z16[idx[128g]]
            #                      + q[128g]*z16[idx[128g-1]]
            # (partition-aligned on 32 partitions; patched into each ot tile's
            # row 0 by a small SBUF->SBUF DMA before the tile's out write)
            haloq = cpool.tile([G, D], F16)
            nc.gpsimd.indirect_dma_start(
                out=haloq[:],
                out_offset=None,
                in_=z_d[:, :],
                in_offset=bass.IndirectOffsetOnAxis(ap=hoff[:, 0:1], axis=0),
            )
            halop = cpool.tile([G, D], F16)
            nc.gpsimd.indirect_dma_start(
                out=halop[:],
                out_offset=None,
                in_=z_d[:, :],
                in_offset=bass.IndirectOffsetOnAxis(ap=hoffp[:, 0:1], axis=0),
            )
            h1 = cpool.tile([G, D], F16)
            nc.vector.tensor_scalar(
                h1[:], halop[:], ph[:, 0:1], None, op0=ALU.mult
            )
            row0c = cpool.tile([G, D], F16)
            nc.vector.scalar_tensor_tensor(
                row0c[:], in0=haloq[:], scalar=qh[:, 0:1], in1=h1[:],
                op0=ALU.mult, op1=ALU.add,
            )

            chunk = None
            for g in range(G):
                # -- gather up[t] = z16[idx[t]] --
                if gather_mode == "dma_gather":
                    j = g % CH
                    if j == 0:
                        c = g // CH
                        ncols = CH * P // 16
                        chunk = upool.tile([P, CH, D], F16)
                        nc.gpsimd.dma_gather(
                            out_ap=chunk[:],
                            in_ap=z_d[:, :],
                            idxs_ap=w16[:, c * ncols : (c + 1) * ncols],
                            num_idxs=CH * P,
                            num_idxs_reg=CH * P,
                            elem_size=D,
                        )
                    up = chunk[:, j, :]
                elif gcols > 1:
                    j = g % gcols
                    if j == 0:
                        c = g // gcols
                        chunk = upool.tile([P, gcols, D], F16)
                        nc.gpsimd.indirect_dma_start(
                            out=chunk[:],
                            out_offset=None,
                            in_=z_d[:, :],
                            in_offset=bass.IndirectOffsetOnAxis(
                                ap=idxc[:, c * gcols : (c + 1) * gcols], axis=0
                            ),
                        )
                    up = chunk[:, j, :]
                else:
                    up_t = upool.tile([P, D], F16)
                    up = up_t[:]
                    nc.gpsimd.indirect_dma_start(
                        out=up,
                        out_offset=None,
                        in_=z_d[:, :],
                        in_offset=bass.IndirectOffsetOnAxis(
                            ap=idxc[:, g : g + 1], axis=0
                        ),
                    )

                # -- build C_g^T = Ssub + (Id - Ssub) * broadcast(p_g) --
                tmp = tmppool.tile([P, P], F16)
                nc.vector.tensor_tensor(
                    tmp[:], dpm[:], pball[:, g * P : (g + 1) * P], ALU.mult
                )
                ct = ctpool.tile([P, P], F16)
                nc.vector.tensor_tensor(ct[:], tmp[:], ssub[:], ALU.add)

                # -- fused roll+blend matmul --
                ps = ppool.tile([P, D], F32)
                for h in range(0, D, 512):
                    nc.tensor.matmul(
                        ps[:, h : h + 512], lhsT=ct[:], rhs=up[:, h : h + 512],
                        start=True, stop=True,
                    )

                # -- PSUM -> SBUF f16, split across ACT and DVE --
                ot = opool.tile([P, D], F16)
                nc.scalar.activation(ot[:, 0:512], ps[:, 0:512], func=ACTF.Copy)
                nc.vector.tensor_copy(ot[:, 512:1024], ps[:, 512:1024])
                # row 0 gets the independently computed correct head value
                # (SBUF->SBUF partition move; deps tracked via ot tile)
                nc.scalar.dma_start(ot[0:1, :], row0c[g : g + 1, :])

                nc.sync.dma_start(out_d[g * P : (g + 1) * P, :], ot[:])


    nc.compile()
    return nc


_NC_CACHE: dict[str, bacc.Bacc] = {}


def get_nc_v2(gcols: int | None = None, gather_mode: str | None = None) -> bacc.Bacc:
    if gcols is None:
        gcols = GCOLS
    if gather_mode is None:
        gather_mode = GATHER_MODE
    key = f"v2:{gcols}:{gather_mode}"
    if key not in _NC_CACHE:
        _NC_CACHE[key] = build_nc_v2(gcols, gather_mode)
    return _NC_CACHE[key]


def make_in_maps_v2(z: np.ndarray, p: np.ndarray, b: np.ndarray) -> list[dict]:
    consts = _const_inputs_v2()
    maps = []
    for i in range(N_CORES):
        bi = b[i].astype(np.int64)
        idx = np.clip(np.cumsum(bi) - bi, 0, LZ - 1).astype(np.int32)
        idx_cm = np.ascontiguousarray(idx.reshape(G, P).T)  # [P, G]
        p16 = p[i].astype(np.float16).reshape(1, T).copy()
        # head rows: out[128g] = p[128g]*z16[idx[128g]] + q[128g]*z16[idx[128g-1]]
        # (for g=0: p=1, q=0 -> out[0] = z16[idx[0]])
        hoff = np.zeros((G, 1), dtype=np.int32)
        hoff[1:, 0] = idx[P - 1 :: P][: G - 1]
        hoffp = np.ascontiguousarray(idx[::P].reshape(G, 1))
        qh = np.zeros((G, 1), dtype=np.float32)
        qh[1:, 0] = (1.0 - p[i, P::P].astype(np.float64)).astype(np.float32)
        ph = p[i, ::P].astype(np.float32).reshape(G, 1).copy()
        ph[0, 0] = 1.0
        p16[0, 0] = 1.0
        m = {
            "z16": z[i].astype(np.float16),
            "p16": p16,
            "idxc": idx_cm,
            "hoff": hoff,
            "hoffp": hoffp,
            "qhc": qh,
            "phc": ph,
        }
        if GATHER_MODE == "dma_gather":
            # wrap-16 int16 layout: W[j, c] = idx[16c + j], replicated to 128
            w16 = np.ascontiguousarray(
                np.tile(idx.reshape(T // 16, 16).T.astype(np.int16), (P // 16, 1))
            )
            m["w16i"] = w16
        m.update(consts)
        maps.append(m)
    return maps


def run(z, p, b, **spmd_kwargs):
    nc = get_nc_v2()
    in_maps = make_in_maps_v2(z, p, b)
    res = run_bass_kernel_spmd(nc, in_maps, core_ids=list(range(N_CORES)), **spmd_kwargs)
    out = np.stack([res.results[i]["out"] for i in range(N_CORES)], axis=0)
    return out, res


def kernel(z, p, b, original_len=None, **_ignored) -> np.ndarray:
    z = np.asarray(z)
    p = np.asarray(p)
    b = np.asarray(b)
    assert z.shape == (N_CORES, LZ, D), z.shape
    assert p.shape == (N_CORES, T), p.shape
    assert b.shape == (N_CORES, T), b.shape
    out, _ = run(z, p, b)
    return out.astype(np.float32, copy=False)


# revision 13
# speedup vs baseline: 3.3886x; 1.0392x over previous
"""Trainium2 Bass kernel for nn_DechunkingLayer.

Full-input contract: kernel(z, p, b, original_len) with
  z [8, 1024, 1024] f32, p [8, 4096] f32, b [8, 4096] i32  ->  [8, 4096, 1024] f32

Sharding: data-parallel over batch — core i processes row i (cumsum / gather /
roll are independent per batch row).

v2 design (fp16 staging, fused shift+blend matmul):
  host:   idx = clip(cumsum(b)-b, 0, Lz-1)  (pure input marshalling)
          z16 = z.astype(f16); output returned as f16 -> f32 host-side.
          Halves device HBM traffic (16.5MB vs 33.7MB) and tunnel bytes.
  device, per 128-row t-tile g:
          up = z16[idx[t]]                 # gpsimd indirect gather, f16 rows
          ps = C_g @ up (+ halo)           # ONE PE matmul: C_g has p on the
                                           # diag and q=1-p on the subdiag, so
                                           # it does roll+blend in one pass;
                                           # the cross-tile halo row is a 2nd
                                           # tiny accumulate-matmul with
                                           # q[128g] at lhsT row 127.
          out[g] = ps (f16)                # ACT+DVE copy PSUM->SBUF, DMA out
  C_g^T is built on-chip: Pb = ones^T @ p_row (PE broadcast), then
  C^T = Ssub + (Id - Ssub) * Pb (2 DVE ops on [128,128] f16).
"""

import numpy as np

import concourse.bass as bass
import concourse.bacc as bacc
import concourse.tile as tile
from concourse import mybir
from concourse.bass_utils import run_bass_kernel_spmd

P = 128       # partitions / t-tile height
G = 32        # t-tiles = T // P
T = 4096
LZ = 1024
D = 1024
N_CORES = 8

F32 = mybir.dt.float32
F16 = mybir.dt.float16
I32 = mybir.dt.int32
ALU = mybir.AluOpType
ACTF = mybir.ActivationFunctionType

GCOLS = 1   # t-tiles gathered per indirect_dma_start (>1 is broken: device abort)
GATHER_MODE = "indirect"   # "indirect" | "dma_gather"
CH = 4      # t-tiles per dma_gather call


def _const_inputs_v2() -> dict[str, np.ndarray]:
    return {
        "ssub": np.eye(P, k=1, dtype=np.float16),   # lhsT[k,t]=1 iff k==t-1
        "dpm": (np.eye(P, dtype=np.float32)
                - np.eye(P, k=1, dtype=np.float32)).astype(np.float16),
        "ones1": np.ones((1, P), dtype=np.float16),
    }


def build_nc_v2(gcols: int | None = None, gather_mode: str | None = None) -> bacc.Bacc:
    if gcols is None:
        gcols = GCOLS
    if gather_mode is None:
        gather_mode = GATHER_MODE
    assert G % gcols == 0
    nc = bacc.Bacc("TRN2", target_bir_lowering=False, debug=False)

    z_d = nc.dram_tensor("z16", [LZ, D], F16, kind="ExternalInput")
    p_d = nc.dram_tensor("p16", [1, T], F16, kind="ExternalInput")
    idx_d = nc.dram_tensor("idxc", [P, G], I32, kind="ExternalInput")
    if gather_mode == "dma_gather":
        w16_d = nc.dram_tensor("w16i", [P, T // 16], mybir.dt.int16,
                               kind="ExternalInput")
    hoff_d = nc.dram_tensor("hoff", [G, 1], I32, kind="ExternalInput")
    hoffp_d = nc.dram_tensor("hoffp", [G, 1], I32, kind="ExternalInput")
    qh_d = nc.dram_tensor("qhc", [G, 1], F32, kind="ExternalInput")
    ph_d = nc.dram_tensor("phc", [G, 1], F32, kind="ExternalInput")
    ssub_d = nc.dram_tensor("ssub", [P, P], F16, kind="ExternalInput")
    dpm_d = nc.dram_tensor("dpm", [P, P], F16, kind="ExternalInput")
    ones1_d = nc.dram_tensor("ones1", [1, P], F16, kind="ExternalInput")
    out_d = nc.dram_tensor("out", [T, D], F16, kind="ExternalOutput")

    with tile.TileContext(nc) as tc:
        with (
            tc.tile_pool(name="consts", bufs=1) as cpool,
            tc.tile_pool(name="up", bufs=8) as upool,
            tc.tile_pool(name="ct", bufs=2) as ctpool,
            tc.tile_pool(name="cttmp", bufs=2) as tmppool,
            tc.tile_pool(name="outp", bufs=4) as opool,
            tc.tile_pool(name="pbps", bufs=1, space="PSUM") as pbpsum,
            tc.tile_pool(name="psum", bufs=3, space="PSUM") as ppool,
        ):
            # ---- constants / small inputs (idxc first: gathers need it) ----
            idxc = cpool.tile([P, G], I32)
            nc.sync.dma_start(idxc[:], idx_d[:, :])
            if gather_mode == "dma_gather":
                w16 = cpool.tile([P, T // 16], mybir.dt.int16)
                nc.sync.dma_start(w16[:], w16_d[:, :])
            ones1 = cpool.tile([1, P], F16)
            nc.sync.dma_start(ones1[:], ones1_d[:, :])
            p16 = cpool.tile([1, T], F16)
            nc.sync.dma_start(p16[:], p_d[:, :])
            ssub = cpool.tile([P, P], F16)
            nc.scalar.dma_start(ssub[:], ssub_d[:, :])
            dpm = cpool.tile([P, P], F16)
            nc.scalar.dma_start(dpm[:], dpm_d[:, :])
            hoff = cpool.tile([G, 1], I32)
            nc.scalar.dma_start(hoff[:], hoff_d[:, :])
            hoffp = cpool.tile([G, 1], I32)
            nc.scalar.dma_start(hoffp[:], hoffp_d[:, :])
            qh = cpool.tile([G, 1], F32)
            nc.scalar.dma_start(qh[:], qh_d[:, :])
            ph = cpool.tile([G, 1], F32)
            nc.scalar.dma_start(ph[:], ph_d[:, :])
            out_head_rows = out_d[:, :].rearrange("(g x) d -> g x d", x=P)[:, 0, :]

            # ---- broadcast p to all partitions once: pball[k, t] = p[t] ----
            pball = cpool.tile([P, T], F16)
            for j in range(T // 512):
                pb_ps = pbpsum.tile([P, 512], F32)
                nc.tensor.matmul(
                    pb_ps[:], lhsT=ones1[:], rhs=p16[0:1, j * 512 : (j + 1) * 512],
                    start=True, stop=True,
                )
                nc.scalar.activation(
                    pball[:, j * 512 : (j + 1) * 512], pb_ps[:], func=ACTF.Copy
                )

            # head rows: row0c[g] = p[128g]*z16[idx[128g]]
            #                      + q[128g]*z16[idx[128g-1]]
            # (partition-aligned on 32 partitions; patched into each ot tile's
            # row 0 by a small SBUF->SBUF DMA before the tile's out write)
            haloq = cpool.tile([G, D], F16)
            nc.gpsimd.indirect_dma_start(
                out=haloq[:],
                out_offset=None,
                in_=z_d[:, :],
                in_offset=bass.IndirectOffsetOnAxis(ap=hoff[:, 0:1], axis=0),
            )
            halop = cpool.tile([G, D], F16)
            nc.gpsimd.indirect_dma_start(
                out=halop[:],
                out_offset=None,
                in_=z_d[:, :],
                in_offset=bass.IndirectOffsetOnAxis(ap=hoffp[:, 0:1], axis=0),
            )
            h1 = cpool.tile([G, D], F16)
            nc.vector.tensor_scalar(
                h1[:], halop[:], ph[:, 0:1], None, op0=ALU.mult
            )
            row0c = cpool.tile([G, D], F16)
            nc.vector.scalar_tensor_tensor(
                row0c[:], in0=haloq[:], scalar=qh[:, 0:1], in1=h1[:],
                op0=ALU.mult, op1=ALU.add,
            )

            chunk = None
            for g in range(G):
                # -- gather up[t] = z16[idx[t]] --
                if gather_mode == "dma_gather":
                    j = g % CH
                    if j == 0:
                        c = g // CH
                        ncols = CH * P // 16
                        chunk = upool.tile([P, CH, D], F16)
                        nc.gpsimd.dma_gather(
                            out_ap=chunk[:],
                            in_ap=z_d[:, :],
                            idxs_ap=w16[:, c * ncols : (c + 1) * ncols],
                            num_idxs=CH * P,
                            num_idxs_reg=CH * P,
                            elem_size=D,
                        )
                    up = chunk[:, j, :]
                elif gcols > 1:
                    j = g % gcols
                    if j == 0:
                        c = g // gcols
                        chunk = upool.tile([P, gcols, D], F16)
                        nc.gpsimd.indirect_dma_start(
                            out=chunk[:],
                            out_offset=None,
                            in_=z_d[:, :],
                            in_offset=bass.IndirectOffsetOnAxis(
                                ap=idxc[:, c * gcols : (c + 1) * gcols], axis=0
                            ),
                        )
                    up = chunk[:, j, :]
                else:
                    up_t = upool.tile([P, D], F16)
                    up = up_t[:]
                    nc.gpsimd.indirect_dma_start(
                        out=up,
                        out_offset=None,
                        in_=z_d[:, :],
                        in_offset=bass.IndirectOffsetOnAxis(
                            ap=idxc[:, g : g + 1], axis=0
                        ),
                    )

                # -- build C_g^T = Ssub + (Id - Ssub) * broadcast(p_g) --
                tmp = tmppool.tile([P, P], F16)
                nc.vector.tensor_tensor(
                    tmp[:], dpm[:], pball[:, g * P : (g + 1) * P], ALU.mult
                )
                ct = ctpool.tile([P, P], F16)
                nc.vector.tensor_tensor(ct[:], tmp[:], ssub[:], ALU.add)

                # -- fused roll+blend matmul --
                ps = ppool.tile([P, D], F32)
                for h in range(0, D, 512):
                    nc.tensor.matmul(
                        ps[:, h : h + 512], lhsT=ct[:], rhs=up[:, h : h + 512],
                        start=True, stop=True,
                    )

                # -- PSUM -> SBUF f16, split across ACT and DVE --
                ot = opool.tile([P, D], F16)
                nc.scalar.activation(ot[:, 0:512], ps[:, 0:512], func=ACTF.Copy)
                nc.vector.tensor_copy(ot[:, 512:1024], ps[:, 512:1024])
                # row 0 gets the independently computed correct head value
                # (SBUF->SBUF partition move; deps tracked via ot tile)
                nc.sync.dma_start(ot[0:1, :], row0c[g : g + 1, :])

                nc.sync.dma_start(out_d[g * P : (g + 1) * P, :], ot[:])


    nc.compile()
    return nc


_NC_CACHE: dict[str, bacc.Bacc] = {}


def get_nc_v2(gcols: int | None = None, gather_mode: str | None = None) -> bacc.Bacc:
    if gcols is None:
        gcols = GCOLS
    if gather_mode is None:
        gather_mode = GATHER_MODE
    key = f"v2:{gcols}:{gather_mode}"
    if key not in _NC_CACHE:
        _NC_CACHE[key] = build_nc_v2(gcols, gather_mode)
    return _NC_CACHE[key]


def make_in_maps_v2(z: np.ndarray, p: np.ndarray, b: np.ndarray) -> list[dict]:
    consts = _const_inputs_v2()
    maps = []
    for i in range(N_CORES):
        bi = b[i].astype(np.int64)
        idx = np.clip(np.cumsum(bi) - bi, 0, LZ - 1).astype(np.int32)
        idx_cm = np.ascontiguousarray(idx.reshape(G, P).T)  # [P, G]
        p16 = p[i].astype(np.float16).reshape(1, T).copy()
        # head rows: out[128g] = p[128g]*z16[idx[128g]] + q[128g]*z16[idx[128g-1]]
        # (for g=0: p=1, q=0 -> out[0] = z16[idx[0]])
        hoff = np.zeros((G, 1), dtype=np.int32)
        hoff[1:, 0] = idx[P - 1 :: P][: G - 1]
        hoffp = np.ascontiguousarray(idx[::P].reshape(G, 1))
        qh = np.zeros((G, 1), dtype=np.float32)
        qh[1:, 0] = (1.0 - p[i, P::P].astype(np.float64)).astype(np.float32)
        ph = p[i, ::P].astype(np.float32).reshape(G, 1).copy()
        ph[0, 0] = 1.0
        p16[0, 0] = 1.0
        m = {
            "z16": z[i].astype(np.float16),
            "p16": p16,
            "idxc": idx_cm,
            "hoff": hoff,
            "hoffp": hoffp,
            "qhc": qh,
            "phc": ph,
        }
        if GATHER_MODE == "dma_gather":
            # wrap-16 int16 layout: W[j, c] = idx[16c + j], replicated to 128
            w16 = np.ascontiguousarray(
                np.tile(idx.reshape(T // 16, 16).T.astype(np.int16), (P // 16, 1))
            )
            m["w16i"] = w16
        m.update(consts)
        maps.append(m)
    return maps


def run(z, p, b, **spmd_kwargs):
    nc = get_nc_v2()
    in_maps = make_in_maps_v2(z, p, b)
    res = run_bass_kernel_spmd(nc, in_maps, core_ids=list(range(N_CORES)), **spmd_kwargs)
    out = np.stack([res.results[i]["out"] for i in range(N_CORES)], axis=0)
    return out, res


def kernel(z, p, b, original_len=None, **_ignored) -> np.ndarray:
    z = np.asarray(z)
    p = np.asarray(p)
    b = np.asarray(b)
    assert z.shape == (N_CORES, LZ, D), z.shape
    assert p.shape == (N_CORES, T), p.shape
    assert b.shape == (N_CORES, T), b.shape
    out, _ = run(z, p, b)
    return out.astype(np.float32, copy=False)


# revision 17
# speedup vs baseline: 4.2046x; 1.2408x over previous
"""Trainium2 Bass kernel for nn_DechunkingLayer.

Full-input contract: kernel(z, p, b, original_len) with
  z [8, 1024, 1024] f32, p [8, 4096] f32, b [8, 4096] i32  ->  [8, 4096, 1024] f32

Sharding: data-parallel over batch — core i processes row i (cumsum / gather /
roll are independent per batch row).

v2 design (fp16 staging, fused shift+blend matmul):
  host:   idx = clip(cumsum(b)-b, 0, Lz-1)  (pure input marshalling)
          z16 = z.astype(f16); output returned as f16 -> f32 host-side.
          Halves device HBM traffic (16.5MB vs 33.7MB) and tunnel bytes.
  device, per 128-row t-tile g:
          up = z16[idx[t]]                 # gpsimd indirect gather, f16 rows
          ps = C_g @ up (+ halo)           # ONE PE matmul: C_g has p on the
                                           # diag and q=1-p on the subdiag, so
                                           # it does roll+blend in one pass;
                                           # the cross-tile halo row is a 2nd
                                           # tiny accumulate-matmul with
                                           # q[128g] at lhsT row 127.
          out[g] = ps (f16)                # ACT+DVE copy PSUM->SBUF, DMA out
  C_g^T is built on-chip: Pb = ones^T @ p_row (PE broadcast), then
  C^T = Ssub + (Id - Ssub) * Pb (2 DVE ops on [128,128] f16).
"""

import numpy as np

import concourse.bass as bass
import concourse.bacc as bacc
import concourse.tile as tile
from concourse import mybir
from concourse.bass_utils import run_bass_kernel_spmd

P = 128       # partitions / t-tile height
G = 32        # t-tiles = T // P
T = 4096
LZ = 1024
D = 1024
N_CORES = 8

F32 = mybir.dt.float32
F16 = mybir.dt.float16
I32 = mybir.dt.int32
ALU = mybir.AluOpType
ACTF = mybir.ActivationFunctionType

GCOLS = 1   # t-tiles gathered per indirect_dma_start (>1 is broken: device abort)
GATHER_MODE = "indirect"   # "indirect" | "dma_gather"
CH = 4      # t-tiles per dma_gather call


def _const_inputs_v2() -> dict[str, np.ndarray]:
    return {
        "ssub": np.eye(P, k=1, dtype=np.float16),   # lhsT[k,t]=1 iff k==t-1
        "dpm": (np.eye(P, dtype=np.float32)
                - np.eye(P, k=1, dtype=np.float32)).astype(np.float16),
        "ones1": np.ones((1, P), dtype=np.float16),
    }


def build_nc_v2(gcols: int | None = None, gather_mode: str | None = None) -> bacc.Bacc:
    if gcols is None:
        gcols = GCOLS
    if gather_mode is None:
        gather_mode = GATHER_MODE
    assert G % gcols == 0
    nc = bacc.Bacc("TRN2", target_bir_lowering=False, debug=False)

    z_d = nc.dram_tensor("z16", [LZ, D], F16, kind="ExternalInput")
    p_d = nc.dram_tensor("p16", [1, T], F16, kind="ExternalInput")
    idx_d = nc.dram_tensor("idxc", [P, G], I32, kind="ExternalInput")
    if gather_mode == "dma_gather":
        w16_d = nc.dram_tensor("w16i", [P, T // 16], mybir.dt.int16,
                               kind="ExternalInput")
    hoff_d = nc.dram_tensor("hoff", [G, 1], I32, kind="ExternalInput")
    hoffp_d = nc.dram_tensor("hoffp", [G, 1], I32, kind="ExternalInput")
    qh_d = nc.dram_tensor("qhc", [G, 1], F32, kind="ExternalInput")
    ph_d = nc.dram_tensor("phc", [G, 1], F32, kind="ExternalInput")
    ssub_d = nc.dram_tensor("ssub", [P, P], F16, kind="ExternalInput")
    dpm_d = nc.dram_tensor("dpm", [P, P], F16, kind="ExternalInput")
    ones1_d = nc.dram_tensor("ones1", [1, P], F16, kind="ExternalInput")
    out_d = nc.dram_tensor("out", [T, D], F16, kind="ExternalOutput")

    with tile.TileContext(nc) as tc:
        with (
            tc.tile_pool(name="consts", bufs=1) as cpool,
            tc.tile_pool(name="up", bufs=12) as upool,
            tc.tile_pool(name="ct", bufs=4) as ctpool,
            tc.tile_pool(name="cttmp", bufs=4) as tmppool,
            tc.tile_pool(name="outp", bufs=6) as opool,
            tc.tile_pool(name="pbps", bufs=1, space="PSUM") as pbpsum,
            tc.tile_pool(name="psum", bufs=3, space="PSUM") as ppool,
        ):
            # ---- constants / small inputs (idxc first: gathers need it) ----
            idxc = cpool.tile([P, G], I32)
            nc.sync.dma_start(idxc[:], idx_d[:, :])
            if gather_mode == "dma_gather":
                w16 = cpool.tile([P, T // 16], mybir.dt.int16)
                nc.sync.dma_start(w16[:], w16_d[:, :])
            ones1 = cpool.tile([1, P], F16)
            nc.sync.dma_start(ones1[:], ones1_d[:, :])
            p16 = cpool.tile([1, T], F16)
            nc.sync.dma_start(p16[:], p_d[:, :])
            ssub = cpool.tile([P, P], F16)
            nc.scalar.dma_start(ssub[:], ssub_d[:, :])
            dpm = cpool.tile([P, P], F16)
            nc.scalar.dma_start(dpm[:], dpm_d[:, :])
            hoff = cpool.tile([G, 1], I32)
            nc.scalar.dma_start(hoff[:], hoff_d[:, :])
            hoffp = cpool.tile([G, 1], I32)
            nc.scalar.dma_start(hoffp[:], hoffp_d[:, :])
            qh = cpool.tile([G, 1], F32)
            nc.scalar.dma_start(qh[:], qh_d[:, :])
            ph = cpool.tile([G, 1], F32)
            nc.scalar.dma_start(ph[:], ph_d[:, :])
            out_head_rows = out_d[:, :].rearrange("(g x) d -> g x d", x=P)[:, 0, :]

            # ---- broadcast p to all partitions once: pball[k, t] = p[t] ----
            pball = cpool.tile([P, T], F16)
            for j in range(T // 512):
                pb_ps = pbpsum.tile([P, 512], F32)
                nc.tensor.matmul(
                    pb_ps[:], lhsT=ones1[:], rhs=p16[0:1, j * 512 : (j + 1) * 512],
                    start=True, stop=True,
                )
                nc.scalar.activation(
                    pball[:, j * 512 : (j + 1) * 512], pb_ps[:], func=ACTF.Copy
                )

            # head rows: row0c[g] = p[128g]*z16[idx[128g]]
            #                      + q[128g]*z16[idx[128g-1]]
            # (partition-aligned on 32 partitions; patched into each ot tile's
            # row 0 by a small SBUF->SBUF DMA before the tile's out write)
            haloq = cpool.tile([G, D], F16)
            nc.gpsimd.indirect_dma_start(
                out=haloq[:],
                out_offset=None,
                in_=z_d[:, :],
                in_offset=bass.IndirectOffsetOnAxis(ap=hoff[:, 0:1], axis=0),
            )
            halop = cpool.tile([G, D], F16)
            nc.gpsimd.indirect_dma_start(
                out=halop[:],
                out_offset=None,
                in_=z_d[:, :],
                in_offset=bass.IndirectOffsetOnAxis(ap=hoffp[:, 0:1], axis=0),
            )
            h1 = cpool.tile([G, D], F16)
            nc.vector.tensor_scalar(
                h1[:], halop[:], ph[:, 0:1], None, op0=ALU.mult
            )
            row0c = cpool.tile([G, D], F16)
            nc.vector.scalar_tensor_tensor(
                row0c[:], in0=haloq[:], scalar=qh[:, 0:1], in1=h1[:],
                op0=ALU.mult, op1=ALU.add,
            )

            chunk = None
            for g in range(G):
                # -- gather up[t] = z16[idx[t]] --
                if gather_mode == "dma_gather":
                    j = g % CH
                    if j == 0:
                        c = g // CH
                        ncols = CH * P // 16
                        chunk = upool.tile([P, CH, D], F16)
                        nc.gpsimd.dma_gather(
                            out_ap=chunk[:],
                            in_ap=z_d[:, :],
                            idxs_ap=w16[:, c * ncols : (c + 1) * ncols],
                            num_idxs=CH * P,
                            num_idxs_reg=CH * P,
                            elem_size=D,
                        )
                    up = chunk[:, j, :]
                elif gcols > 1:
                    j = g % gcols
                    if j == 0:
                        c = g // gcols
                        chunk = upool.tile([P, gcols, D], F16)
                        nc.gpsimd.indirect_dma_start(
                            out=chunk[:],
                            out_offset=None,
                            in_=z_d[:, :],
                            in_offset=bass.IndirectOffsetOnAxis(
                                ap=idxc[:, c * gcols : (c + 1) * gcols], axis=0
                            ),
                        )
                    up = chunk[:, j, :]
                else:
                    up_t = upool.tile([P, D], F16)
                    up = up_t[:]
                    nc.gpsimd.indirect_dma_start(
                        out=up,
                        out_offset=None,
                        in_=z_d[:, :],
                        in_offset=bass.IndirectOffsetOnAxis(
                            ap=idxc[:, g : g + 1], axis=0
                        ),
                    )

                # -- build C_g^T = Ssub + (Id - Ssub) * broadcast(p_g) --
                tmp = tmppool.tile([P, P], F16)
                nc.vector.tensor_tensor(
                    tmp[:], dpm[:], pball[:, g * P : (g + 1) * P], ALU.mult
                )
                ct = ctpool.tile([P, P], F16)
                nc.vector.tensor_tensor(ct[:], tmp[:], ssub[:], ALU.add)

                # -- fused roll+blend matmul --
                ps = ppool.tile([P, D], F32)
                for h in range(0, D, 512):
                    nc.tensor.matmul(
                        ps[:, h : h + 512], lhsT=ct[:], rhs=up[:, h : h + 512],
                        start=True, stop=True,
                    )

                # -- PSUM -> SBUF f16, split across ACT and DVE --
                ot = opool.tile([P, D], F16)
                nc.scalar.activation(ot[:, 0:512], ps[:, 0:512], func=ACTF.Copy)
                nc.vector.tensor_copy(ot[:, 512:1024], ps[:, 512:1024])
                nc.sync.dma_start(out_d[g * P : (g + 1) * P, :], ot[:])


    nc.compile()
    return nc


_NC_CACHE: dict[str, bacc.Bacc] = {}


def get_nc_v2(gcols: int | None = None, gather_mode: str | None = None) -> bacc.Bacc:
    if gcols is None:
        gcols = GCOLS
    if gather_mode is None:
        gather_mode = GATHER_MODE
    key = f"v2:{gcols}:{gather_mode}"
    if key not in _NC_CACHE:
        _NC_CACHE[key] = build_nc_v2(gcols, gather_mode)
    return _NC_CACHE[key]


def make_in_maps_v2(z: np.ndarray, p: np.ndarray, b: np.ndarray) -> list[dict]:
    consts = _const_inputs_v2()
    maps = []
    for i in range(N_CORES):
        bi = b[i].astype(np.int64)
        idx = np.clip(np.cumsum(bi) - bi, 0, LZ - 1).astype(np.int32)
        idx_cm = np.ascontiguousarray(idx.reshape(G, P).T)  # [P, G]
        p16 = p[i].astype(np.float16).reshape(1, T).copy()
        # head rows: out[128g] = p[128g]*z16[idx[128g]] + q[128g]*z16[idx[128g-1]]
        # (for g=0: p=1, q=0 -> out[0] = z16[idx[0]])
        hoff = np.zeros((G, 1), dtype=np.int32)
        hoff[1:, 0] = idx[P - 1 :: P][: G - 1]
        hoffp = np.ascontiguousarray(idx[::P].reshape(G, 1))
        qh = np.zeros((G, 1), dtype=np.float32)
        qh[1:, 0] = (1.0 - p[i, P::P].astype(np.float64)).astype(np.float32)
        ph = p[i, ::P].astype(np.float32).reshape(G, 1).copy()
        ph[0, 0] = 1.0
        p16[0, 0] = 1.0
        m = {
            "z16": z[i].astype(np.float16),
            "p16": p16,
            "idxc": idx_cm,
            "hoff": hoff,
            "hoffp": hoffp,
            "qhc": qh,
            "phc": ph,
        }
        if GATHER_MODE == "dma_gather":
            # wrap-16 int16 layout: W[j, c] = idx[16c + j], replicated to 128
            w16 = np.ascontiguousarray(
                np.tile(idx.reshape(T // 16, 16).T.astype(np.int16), (P // 16, 1))
            )
            m["w16i"] = w16
        m.update(consts)
        maps.append(m)
    return maps


def run(z, p, b, **spmd_kwargs):
    nc = get_nc_v2()
    in_maps = make_in_maps_v2(z, p, b)
    res = run_bass_kernel_spmd(nc, in_maps, core_ids=list(range(N_CORES)), **spmd_kwargs)
    out = np.stack([res.results[i]["out"] for i in range(N_CORES)], axis=0)
    return out, res


KERNEL_V = "v3"   # "v2" | "v3"


def kernel(z, p, b, original_len=None, **_ignored) -> np.ndarray:
    z = np.asarray(z)
    p = np.asarray(p)
    b = np.asarray(b)
    assert z.shape == (N_CORES, LZ, D), z.shape
    assert p.shape == (N_CORES, T), p.shape
    assert b.shape == (N_CORES, T), b.shape
    runner = run_v3 if KERNEL_V == "v3" else run
    out, _ = runner(z, p, b)
    return out.astype(np.float32, copy=False)

# ---------------------------------------------------------------------------
# v3: overlapping stride-127 tiles - the roll halo is INSIDE each tile's
# gather, so there are no cross-tile deps, no head-row fixups, and no DRAM
# WAW. Each tile g gathers up[k] = z16[idx[127g - 1 + k]] (k=0..127) and
# computes out[127g + m] = p*up[m+1] + q*up[m] for m=0..126 via one matmul
# with C'[k,m] = p'[m]@[k==m+1] + q'[m]@[k==m] = Id + (Slow - Id) * Pb.
# Tiles write disjoint aligned 128-row blocks of a [33*128, D] staging
# output; the host strips row 127 of each block.
# ---------------------------------------------------------------------------
G3 = 33  # ceil(T / 127)


def build_nc_v3() -> bacc.Bacc:
    nc = bacc.Bacc("TRN2", target_bir_lowering=False, debug=False)

    TP = G3 * P  # padded output rows / pov length
    z_d = nc.dram_tensor("z16", [LZ, D], F16, kind="ExternalInput")
    p_d = nc.dram_tensor("pov", [1, TP], F16, kind="ExternalInput")
    idx_d = nc.dram_tensor("idxc", [P, G3], I32, kind="ExternalInput")
    id_d = nc.dram_tensor("id128", [P, P], F16, kind="ExternalInput")
    dpm_d = nc.dram_tensor("dpm2", [P, P], F16, kind="ExternalInput")
    ones1_d = nc.dram_tensor("ones1", [1, P], F16, kind="ExternalInput")
    out_d = nc.dram_tensor("out", [TP, D], F16, kind="ExternalOutput")

    with tile.TileContext(nc) as tc:
        with (
            tc.tile_pool(name="consts", bufs=1) as cpool,
            tc.tile_pool(name="up", bufs=12) as upool,
            tc.tile_pool(name="ct", bufs=4) as ctpool,
            tc.tile_pool(name="cttmp", bufs=4) as tmppool,
            tc.tile_pool(name="outp", bufs=6) as opool,
            tc.tile_pool(name="pbps", bufs=1, space="PSUM") as pbpsum,
            tc.tile_pool(name="psum", bufs=3, space="PSUM") as ppool,
        ):
            idxc = cpool.tile([P, G3], I32)
            nc.sync.dma_start(idxc[:], idx_d[:, :])
            ones1 = cpool.tile([1, P], F16)
            nc.sync.dma_start(ones1[:], ones1_d[:, :])
            pov = cpool.tile([1, TP], F16)
            nc.sync.dma_start(pov[:], p_d[:, :])
            id128 = cpool.tile([P, P], F16)
            nc.scalar.dma_start(id128[:], id_d[:, :])
            dpm2 = cpool.tile([P, P], F16)
            nc.scalar.dma_start(dpm2[:], dpm_d[:, :])

            # broadcast pov to all partitions once: pball[k, t] = pov[t]
            pball = cpool.tile([P, TP], F16)
            for j in range(0, TP, 512):
                w = min(512, TP - j)
                pb_ps = pbpsum.tile([P, 512], F32)
                nc.tensor.matmul(
                    pb_ps[:, 0:w], lhsT=ones1[:], rhs=pov[0:1, j : j + w],
                    start=True, stop=True,
                )
                nc.scalar.activation(
                    pball[:, j : j + w], pb_ps[:, 0:w], func=ACTF.Copy
                )

            for g in range(G3):
                up = upool.tile([P, D], F16)
                nc.gpsimd.indirect_dma_start(
                    out=up[:],
                    out_offset=None,
                    in_=z_d[:, :],
                    in_offset=bass.IndirectOffsetOnAxis(
                        ap=idxc[:, g : g + 1], axis=0
                    ),
                )

                tmp = tmppool.tile([P, P], F16)
                nc.vector.tensor_tensor(
                    tmp[:], dpm2[:], pball[:, g * P : (g + 1) * P], ALU.mult
                )
                ct = ctpool.tile([P, P], F16)
                nc.vector.tensor_tensor(ct[:], tmp[:], id128[:], ALU.add)

                ps = ppool.tile([P, D], F32)
                for h in range(0, D, 512):
                    nc.tensor.matmul(
                        ps[:, h : h + 512], lhsT=ct[:], rhs=up[:, h : h + 512],
                        start=True, stop=True,
                    )

                ot = opool.tile([P, D], F16)
                nc.scalar.activation(ot[:, 0:512], ps[:, 0:512], func=ACTF.Copy)
                nc.vector.tensor_copy(ot[:, 512:1024], ps[:, 512:1024])

                nc.sync.dma_start(out_d[g * P : (g + 1) * P, :], ot[:])

    nc.compile()
    return nc


def _const_inputs_v3() -> dict[str, np.ndarray]:
    slow = np.eye(P, k=-1, dtype=np.float32)   # [k==m+1]
    iden = np.eye(P, dtype=np.float32)
    return {
        "id128": iden.astype(np.float16),
        "dpm2": (slow - iden).astype(np.float16),
        "ones1": np.ones((1, P), dtype=np.float16),
    }


def make_in_maps_v3(z: np.ndarray, p: np.ndarray, b: np.ndarray) -> list[dict]:
    consts = _const_inputs_v3()
    TP = G3 * P
    maps = []
    for i in range(N_CORES):
        bi = b[i].astype(np.int64)
        idx = np.clip(np.cumsum(bi) - bi, 0, LZ - 1).astype(np.int32)
        # idx_cm[k, g] = idx[127g - 1 + k]  (clipped; g=0,k=0 dummy killed by q=0)
        kk = np.arange(P)[:, None]
        gg = np.arange(G3)[None, :]
        src = np.clip(127 * gg - 1 + kk, 0, T - 1)
        idx_cm = np.ascontiguousarray(idx[src])
        # pov[128g + m] = p[127g + m] for m<127; col 127 of each block = 0
        pov = np.zeros(TP, dtype=np.float32)
        t_src = (127 * gg + np.arange(P)[:, None]).T  # [G3, P]
        valid = (np.arange(P)[None, :] < 127) & (t_src < T)
        pv = np.where(valid, p[i][np.clip(t_src, 0, T - 1)], 0.0)
        pov = pv.reshape(-1).astype(np.float32)
        pov[0] = 1.0  # out[0] = z[idx[0]] exactly
        m = {
            "z16": z[i].astype(np.float16),
            "pov": pov.astype(np.float16).reshape(1, TP),
            "idxc": idx_cm,
        }
        m.update(consts)
        maps.append(m)
    return maps


def run_v3(z, p, b, **spmd_kwargs):
    key = "v3"
    if key not in _NC_CACHE:
        _NC_CACHE[key] = build_nc_v3()
    nc = _NC_CACHE[key]
    in_maps = make_in_maps_v3(z, p, b)
    res = run_bass_kernel_spmd(nc, in_maps, core_ids=list(range(N_CORES)), **spmd_kwargs)
    outs = []
    for i in range(N_CORES):
        st = res.results[i]["out"].reshape(G3, P, D)[:, : P - 1, :]
        outs.append(st.reshape(-1, D)[:T])
    return np.stack(outs, axis=0), res


# revision 19
# speedup vs baseline: 4.5192x; 1.0748x over previous
"""Trainium2 Bass kernel for nn_DechunkingLayer.

Full-input contract: kernel(z, p, b, original_len) with
  z [8, 1024, 1024] f32, p [8, 4096] f32, b [8, 4096] i32  ->  [8, 4096, 1024] f32

Sharding: data-parallel over batch — core i processes row i (cumsum / gather /
roll are independent per batch row).

v2 design (fp16 staging, fused shift+blend matmul):
  host:   idx = clip(cumsum(b)-b, 0, Lz-1)  (pure input marshalling)
          z16 = z.astype(f16); output returned as f16 -> f32 host-side.
          Halves device HBM traffic (16.5MB vs 33.7MB) and tunnel bytes.
  device, per 128-row t-tile g:
          up = z16[idx[t]]                 # gpsimd indirect gather, f16 rows
          ps = C_g @ up (+ halo)           # ONE PE matmul: C_g has p on the
                                           # diag and q=1-p on the subdiag, so
                                           # it does roll+blend in one pass;
                                           # the cross-tile halo row is a 2nd
                                           # tiny accumulate-matmul with
                                           # q[128g] at lhsT row 127.
          out[g] = ps (f16)                # ACT+DVE copy PSUM->SBUF, DMA out
  C_g^T is built on-chip: Pb = ones^T @ p_row (PE broadcast), then
  C^T = Ssub + (Id - Ssub) * Pb (2 DVE ops on [128,128] f16).
"""

import numpy as np

import concourse.bass as bass
import concourse.bacc as bacc
import concourse.tile as tile
from concourse import mybir
from concourse.bass_utils import run_bass_kernel_spmd

P = 128       # partitions / t-tile height
G = 32        # t-tiles = T // P
T = 4096
LZ = 1024
D = 1024
N_CORES = 8

F32 = mybir.dt.float32
F16 = mybir.dt.float16
I32 = mybir.dt.int32
ALU = mybir.AluOpType
ACTF = mybir.ActivationFunctionType

GCOLS = 1   # t-tiles gathered per indirect_dma_start (>1 is broken: device abort)
GATHER_MODE = "indirect"   # "indirect" | "dma_gather"
CH = 4      # t-tiles per dma_gather call


def _const_inputs_v2() -> dict[str, np.ndarray]:
    return {
        "ssub": np.eye(P, k=1, dtype=np.float16),   # lhsT[k,t]=1 iff k==t-1
        "dpm": (np.eye(P, dtype=np.float32)
                - np.eye(P, k=1, dtype=np.float32)).astype(np.float16),
        "ones1": np.ones((1, P), dtype=np.float16),
    }


def build_nc_v2(gcols: int | None = None, gather_mode: str | None = None) -> bacc.Bacc:
    if gcols is None:
        gcols = GCOLS
    if gather_mode is None:
        gather_mode = GATHER_MODE
    assert G % gcols == 0
    nc = bacc.Bacc("TRN2", target_bir_lowering=False, debug=False)

    z_d = nc.dram_tensor("z16", [LZ, D], F16, kind="ExternalInput")
    p_d = nc.dram_tensor("p16", [1, T], F16, kind="ExternalInput")
    idx_d = nc.dram_tensor("idxc", [P, G], I32, kind="ExternalInput")
    if gather_mode == "dma_gather":
        w16_d = nc.dram_tensor("w16i", [P, T // 16], mybir.dt.int16,
                               kind="ExternalInput")
    hoff_d = nc.dram_tensor("hoff", [G, 1], I32, kind="ExternalInput")
    hoffp_d = nc.dram_tensor("hoffp", [G, 1], I32, kind="ExternalInput")
    qh_d = nc.dram_tensor("qhc", [G, 1], F32, kind="ExternalInput")
    ph_d = nc.dram_tensor("phc", [G, 1], F32, kind="ExternalInput")
    ssub_d = nc.dram_tensor("ssub", [P, P], F16, kind="ExternalInput")
    dpm_d = nc.dram_tensor("dpm", [P, P], F16, kind="ExternalInput")
    ones1_d = nc.dram_tensor("ones1", [1, P], F16, kind="ExternalInput")
    out_d = nc.dram_tensor("out", [T, D], F16, kind="ExternalOutput")

    with tile.TileContext(nc) as tc:
        with (
            tc.tile_pool(name="consts", bufs=1) as cpool,
            tc.tile_pool(name="up", bufs=12) as upool,
            tc.tile_pool(name="ct", bufs=4) as ctpool,
            tc.tile_pool(name="cttmp", bufs=4) as tmppool,
            tc.tile_pool(name="outp", bufs=6) as opool,
            tc.tile_pool(name="pbps", bufs=1, space="PSUM") as pbpsum,
            tc.tile_pool(name="psum", bufs=3, space="PSUM") as ppool,
        ):
            # ---- constants / small inputs (idxc first: gathers need it) ----
            idxc = cpool.tile([P, G], I32)
            nc.sync.dma_start(idxc[:], idx_d[:, :])
            if gather_mode == "dma_gather":
                w16 = cpool.tile([P, T // 16], mybir.dt.int16)
                nc.sync.dma_start(w16[:], w16_d[:, :])
            ones1 = cpool.tile([1, P], F16)
            nc.sync.dma_start(ones1[:], ones1_d[:, :])
            p16 = cpool.tile([1, T], F16)
            nc.sync.dma_start(p16[:], p_d[:, :])
            ssub = cpool.tile([P, P], F16)
            nc.scalar.dma_start(ssub[:], ssub_d[:, :])
            dpm = cpool.tile([P, P], F16)
            nc.scalar.dma_start(dpm[:], dpm_d[:, :])
            hoff = cpool.tile([G, 1], I32)
            nc.scalar.dma_start(hoff[:], hoff_d[:, :])
            hoffp = cpool.tile([G, 1], I32)
            nc.scalar.dma_start(hoffp[:], hoffp_d[:, :])
            qh = cpool.tile([G, 1], F32)
            nc.scalar.dma_start(qh[:], qh_d[:, :])
            ph = cpool.tile([G, 1], F32)
            nc.scalar.dma_start(ph[:], ph_d[:, :])
            out_head_rows = out_d[:, :].rearrange("(g x) d -> g x d", x=P)[:, 0, :]

            # ---- broadcast p to all partitions once: pball[k, t] = p[t] ----
            pball = cpool.tile([P, T], F16)
            for j in range(T // 512):
                pb_ps = pbpsum.tile([P, 512], F32)
                nc.tensor.matmul(
                    pb_ps[:], lhsT=ones1[:], rhs=p16[0:1, j * 512 : (j + 1) * 512],
                    start=True, stop=True,
                )
                nc.scalar.activation(
                    pball[:, j * 512 : (j + 1) * 512], pb_ps[:], func=ACTF.Copy
                )

            # head rows: row0c[g] = p[128g]*z16[idx[128g]]
            #                      + q[128g]*z16[idx[128g-1]]
            # (partition-aligned on 32 partitions; patched into each ot tile's
            # row 0 by a small SBUF->SBUF DMA before the tile's out write)
            haloq = cpool.tile([G, D], F16)
            nc.gpsimd.indirect_dma_start(
                out=haloq[:],
                out_offset=None,
                in_=z_d[:, :],
                in_offset=bass.IndirectOffsetOnAxis(ap=hoff[:, 0:1], axis=0),
            )
            halop = cpool.tile([G, D], F16)
            nc.gpsimd.indirect_dma_start(
                out=halop[:],
                out_offset=None,
                in_=z_d[:, :],
                in_offset=bass.IndirectOffsetOnAxis(ap=hoffp[:, 0:1], axis=0),
            )
            h1 = cpool.tile([G, D], F16)
            nc.vector.tensor_scalar(
                h1[:], halop[:], ph[:, 0:1], None, op0=ALU.mult
            )
            row0c = cpool.tile([G, D], F16)
            nc.vector.scalar_tensor_tensor(
                row0c[:], in0=haloq[:], scalar=qh[:, 0:1], in1=h1[:],
                op0=ALU.mult, op1=ALU.add,
            )

            chunk = None
            for g in range(G):
                # -- gather up[t] = z16[idx[t]] --
                if gather_mode == "dma_gather":
                    j = g % CH
                    if j == 0:
                        c = g // CH
                        ncols = CH * P // 16
                        chunk = upool.tile([P, CH, D], F16)
                        nc.gpsimd.dma_gather(
                            out_ap=chunk[:],
                            in_ap=z_d[:, :],
                            idxs_ap=w16[:, c * ncols : (c + 1) * ncols],
                            num_idxs=CH * P,
                            num_idxs_reg=CH * P,
                            elem_size=D,
                        )
                    up = chunk[:, j, :]
                elif gcols > 1:
                    j = g % gcols
                    if j == 0:
                        c = g // gcols
                        chunk = upool.tile([P, gcols, D], F16)
                        nc.gpsimd.indirect_dma_start(
                            out=chunk[:],
                            out_offset=None,
                            in_=z_d[:, :],
                            in_offset=bass.IndirectOffsetOnAxis(
                                ap=idxc[:, c * gcols : (c + 1) * gcols], axis=0
                            ),
                        )
                    up = chunk[:, j, :]
                else:
                    up_t = upool.tile([P, D], F16)
                    up = up_t[:]
                    nc.gpsimd.indirect_dma_start(
                        out=up,
                        out_offset=None,
                        in_=z_d[:, :],
                        in_offset=bass.IndirectOffsetOnAxis(
                            ap=idxc[:, g : g + 1], axis=0
                        ),
                    )

                # -- build C_g^T = Ssub + (Id - Ssub) * broadcast(p_g) --
                tmp = tmppool.tile([P, P], F16)
                nc.vector.tensor_tensor(
                    tmp[:], dpm[:], pball[:, g * P : (g + 1) * P], ALU.mult
                )
                ct = ctpool.tile([P, P], F16)
                nc.vector.tensor_tensor(ct[:], tmp[:], ssub[:], ALU.add)

                # -- fused roll+blend matmul --
                ps = ppool.tile([P, D], F32)
                for h in range(0, D, 512):
                    nc.tensor.matmul(
                        ps[:, h : h + 512], lhsT=ct[:], rhs=up[:, h : h + 512],
                        start=True, stop=True,
                    )

                # -- PSUM -> SBUF f16, split across ACT and DVE --
                ot = opool.tile([P, D], F16)
                nc.scalar.activation(ot[:, 0:512], ps[:, 0:512], func=ACTF.Copy)
                nc.vector.tensor_copy(ot[:, 512:1024], ps[:, 512:1024])
                nc.sync.dma_start(out_d[g * P : (g + 1) * P, :], ot[:])


    nc.compile()
    return nc


_NC_CACHE: dict[str, bacc.Bacc] = {}


def get_nc_v2(gcols: int | None = None, gather_mode: str | None = None) -> bacc.Bacc:
    if gcols is None:
        gcols = GCOLS
    if gather_mode is None:
        gather_mode = GATHER_MODE
    key = f"v2:{gcols}:{gather_mode}"
    if key not in _NC_CACHE:
        _NC_CACHE[key] = build_nc_v2(gcols, gather_mode)
    return _NC_CACHE[key]


def make_in_maps_v2(z: np.ndarray, p: np.ndarray, b: np.ndarray) -> list[dict]:
    consts = _const_inputs_v2()
    maps = []
    for i in range(N_CORES):
        bi = b[i].astype(np.int64)
        idx = np.clip(np.cumsum(bi) - bi, 0, LZ - 1).astype(np.int32)
        idx_cm = np.ascontiguousarray(idx.reshape(G, P).T)  # [P, G]
        p16 = p[i].astype(np.float16).reshape(1, T).copy()
        # head rows: out[128g] = p[128g]*z16[idx[128g]] + q[128g]*z16[idx[128g-1]]
        # (for g=0: p=1, q=0 -> out[0] = z16[idx[0]])
        hoff = np.zeros((G, 1), dtype=np.int32)
        hoff[1:, 0] = idx[P - 1 :: P][: G - 1]
        hoffp = np.ascontiguousarray(idx[::P].reshape(G, 1))
        qh = np.zeros((G, 1), dtype=np.float32)
        qh[1:, 0] = (1.0 - p[i, P::P].astype(np.float64)).astype(np.float32)
        ph = p[i, ::P].astype(np.float32).reshape(G, 1).copy()
        ph[0, 0] = 1.0
        p16[0, 0] = 1.0
        m = {
            "z16": z[i].astype(np.float16),
            "p16": p16,
            "idxc": idx_cm,
            "hoff": hoff,
            "hoffp": hoffp,
            "qhc": qh,
            "phc": ph,
        }
        if GATHER_MODE == "dma_gather":
            # wrap-16 int16 layout: W[j, c] = idx[16c + j], replicated to 128
            w16 = np.ascontiguousarray(
                np.tile(idx.reshape(T // 16, 16).T.astype(np.int16), (P // 16, 1))
            )
            m["w16i"] = w16
        m.update(consts)
        maps.append(m)
    return maps


def run(z, p, b, **spmd_kwargs):
    nc = get_nc_v2()
    in_maps = make_in_maps_v2(z, p, b)
    res = run_bass_kernel_spmd(nc, in_maps, core_ids=list(range(N_CORES)), **spmd_kwargs)
    out = np.stack([res.results[i]["out"] for i in range(N_CORES)], axis=0)
    return out, res


KERNEL_V = "v4"   # "v2" | "v3" | "v4"


def kernel(z, p, b, original_len=None, **_ignored) -> np.ndarray:
    z = np.asarray(z)
    p = np.asarray(p)
    b = np.asarray(b)
    assert z.shape == (N_CORES, LZ, D), z.shape
    assert p.shape == (N_CORES, T), p.shape
    assert b.shape == (N_CORES, T), b.shape
    runner = {"v3": run_v3, "v4": run_v4}.get(KERNEL_V, run)
    out, _ = runner(z, p, b)
    return out.astype(np.float32, copy=False)

# ---------------------------------------------------------------------------
# v3: overlapping stride-127 tiles - the roll halo is INSIDE each tile's
# gather, so there are no cross-tile deps, no head-row fixups, and no DRAM
# WAW. Each tile g gathers up[k] = z16[idx[127g - 1 + k]] (k=0..127) and
# computes out[127g + m] = p*up[m+1] + q*up[m] for m=0..126 via one matmul
# with C'[k,m] = p'[m]@[k==m+1] + q'[m]@[k==m] = Id + (Slow - Id) * Pb.
# Tiles write disjoint aligned 128-row blocks of a [33*128, D] staging
# output; the host strips row 127 of each block.
# ---------------------------------------------------------------------------
G3 = 33  # ceil(T / 127)


def build_nc_v3() -> bacc.Bacc:
    nc = bacc.Bacc("TRN2", target_bir_lowering=False, debug=False)

    TP = G3 * P  # padded output rows / pov length
    z_d = nc.dram_tensor("z16", [LZ, D], F16, kind="ExternalInput")
    p_d = nc.dram_tensor("pov", [1, TP], F16, kind="ExternalInput")
    idx_d = nc.dram_tensor("idxc", [P, G3], I32, kind="ExternalInput")
    id_d = nc.dram_tensor("id128", [P, P], F16, kind="ExternalInput")
    dpm_d = nc.dram_tensor("dpm2", [P, P], F16, kind="ExternalInput")
    ones1_d = nc.dram_tensor("ones1", [1, P], F16, kind="ExternalInput")
    out_d = nc.dram_tensor("out", [TP, D], F16, kind="ExternalOutput")

    with tile.TileContext(nc) as tc:
        with (
            tc.tile_pool(name="consts", bufs=1) as cpool,
            tc.tile_pool(name="up", bufs=12) as upool,
            tc.tile_pool(name="ct", bufs=4) as ctpool,
            tc.tile_pool(name="cttmp", bufs=4) as tmppool,
            tc.tile_pool(name="outp", bufs=6) as opool,
            tc.tile_pool(name="pbps", bufs=1, space="PSUM") as pbpsum,
            tc.tile_pool(name="psum", bufs=3, space="PSUM") as ppool,
        ):
            idxc = cpool.tile([P, G3], I32)
            nc.sync.dma_start(idxc[:], idx_d[:, :])
            ones1 = cpool.tile([1, P], F16)
            nc.sync.dma_start(ones1[:], ones1_d[:, :])
            pov = cpool.tile([1, TP], F16)
            nc.sync.dma_start(pov[:], p_d[:, :])
            id128 = cpool.tile([P, P], F16)
            nc.scalar.dma_start(id128[:], id_d[:, :])
            dpm2 = cpool.tile([P, P], F16)
            nc.scalar.dma_start(dpm2[:], dpm_d[:, :])

            # broadcast pov to all partitions once: pball[k, t] = pov[t]
            pball = cpool.tile([P, TP], F16)
            for j in range(0, TP, 512):
                w = min(512, TP - j)
                pb_ps = pbpsum.tile([P, 512], F32)
                nc.tensor.matmul(
                    pb_ps[:, 0:w], lhsT=ones1[:], rhs=pov[0:1, j : j + w],
                    start=True, stop=True,
                )
                nc.scalar.activation(
                    pball[:, j : j + w], pb_ps[:, 0:w], func=ACTF.Copy
                )

            for g in range(G3):
                up = upool.tile([P, D], F16)
                nc.gpsimd.indirect_dma_start(
                    out=up[:],
                    out_offset=None,
                    in_=z_d[:, :],
                    in_offset=bass.IndirectOffsetOnAxis(
                        ap=idxc[:, g : g + 1], axis=0
                    ),
                )

                tmp = tmppool.tile([P, P], F16)
                nc.vector.tensor_tensor(
                    tmp[:], dpm2[:], pball[:, g * P : (g + 1) * P], ALU.mult
                )
                ct = ctpool.tile([P, P], F16)
                nc.vector.tensor_tensor(ct[:], tmp[:], id128[:], ALU.add)

                ps = ppool.tile([P, D], F32)
                for h in range(0, D, 512):
                    nc.tensor.matmul(
                        ps[:, h : h + 512], lhsT=ct[:], rhs=up[:, h : h + 512],
                        start=True, stop=True,
                    )

                ot = opool.tile([P, D], F16)
                nc.scalar.activation(ot[:, 0:512], ps[:, 0:512], func=ACTF.Copy)
                nc.vector.tensor_copy(ot[:, 512:1024], ps[:, 512:1024])

                nc.sync.dma_start(out_d[g * P : (g + 1) * P, :], ot[:])

    nc.compile()
    return nc


def _const_inputs_v3() -> dict[str, np.ndarray]:
    slow = np.eye(P, k=-1, dtype=np.float32)   # [k==m+1]
    iden = np.eye(P, dtype=np.float32)
    return {
        "id128": iden.astype(np.float16),
        "dpm2": (slow - iden).astype(np.float16),
        "ones1": np.ones((1, P), dtype=np.float16),
    }


def make_in_maps_v3(z: np.ndarray, p: np.ndarray, b: np.ndarray) -> list[dict]:
    consts = _const_inputs_v3()
    TP = G3 * P
    maps = []
    for i in range(N_CORES):
        bi = b[i].astype(np.int64)
        idx = np.clip(np.cumsum(bi) - bi, 0, LZ - 1).astype(np.int32)
        # idx_cm[k, g] = idx[127g - 1 + k]  (clipped; g=0,k=0 dummy killed by q=0)
        kk = np.arange(P)[:, None]
        gg = np.arange(G3)[None, :]
        src = np.clip(127 * gg - 1 + kk, 0, T - 1)
        idx_cm = np.ascontiguousarray(idx[src])
        # pov[128g + m] = p[127g + m] for m<127; col 127 of each block = 0
        pov = np.zeros(TP, dtype=np.float32)
        t_src = (127 * gg + np.arange(P)[:, None]).T  # [G3, P]
        valid = (np.arange(P)[None, :] < 127) & (t_src < T)
        pv = np.where(valid, p[i][np.clip(t_src, 0, T - 1)], 0.0)
        pov = pv.reshape(-1).astype(np.float32)
        pov[0] = 1.0  # out[0] = z[idx[0]] exactly
        m = {
            "z16": z[i].astype(np.float16),
            "pov": pov.astype(np.float16).reshape(1, TP),
            "idxc": idx_cm,
        }
        m.update(consts)
        maps.append(m)
    return maps


def run_v3(z, p, b, **spmd_kwargs):
    key = "v3"
    if key not in _NC_CACHE:
        _NC_CACHE[key] = build_nc_v3()
    nc = _NC_CACHE[key]
    in_maps = make_in_maps_v3(z, p, b)
    res = run_bass_kernel_spmd(nc, in_maps, core_ids=list(range(N_CORES)), **spmd_kwargs)
    outs = []
    for i in range(N_CORES):
        st = res.results[i]["out"].reshape(G3, P, D)[:, : P - 1, :]
        outs.append(st.reshape(-1, D)[:T])
    return np.stack(outs, axis=0), res

# ---------------------------------------------------------------------------
# v4: like v3 but the gather reads each tile's CONTIGUOUS span of z rows
# (offsets a_g + k, a_g = idx[127g-1]) so HBM reads are sequential and
# SDMA packets aggregate; the data-dependent expansion is folded into the
# blend matrix CT_all[k, 128g+m] = p'[m]*[k==l(m+1)] + q'[m]*[k==l(m)]
# (l = idx - a_g), which the host builds from p and idx and ships (1MB).
# ---------------------------------------------------------------------------


def build_nc_v4() -> bacc.Bacc:
    nc = bacc.Bacc("TRN2", target_bir_lowering=False, debug=False)

    TP = G3 * P
    z_d = nc.dram_tensor("z16", [LZ, D], F16, kind="ExternalInput")
    ct_d = nc.dram_tensor("ctall", [P, TP], F16, kind="ExternalInput")
    idx_d = nc.dram_tensor("idxc", [P, G3], I32, kind="ExternalInput")
    out_d = nc.dram_tensor("out", [TP, D], F16, kind="ExternalOutput")

    with tile.TileContext(nc) as tc:
        with (
            tc.tile_pool(name="consts", bufs=1) as cpool,
            tc.tile_pool(name="up", bufs=12) as upool,
            tc.tile_pool(name="outp", bufs=6) as opool,
            tc.tile_pool(name="psum", bufs=3, space="PSUM") as ppool,
        ):
            idxc = cpool.tile([P, G3], I32)
            nc.sync.dma_start(idxc[:], idx_d[:, :])
            ctall = cpool.tile([P, TP], F16)
            nc.scalar.dma_start(ctall[:], ct_d[:, :])

            for g in range(G3):
                up = upool.tile([P, D], F16)
                nc.gpsimd.indirect_dma_start(
                    out=up[:],
                    out_offset=None,
                    in_=z_d[:, :],
                    in_offset=bass.IndirectOffsetOnAxis(
                        ap=idxc[:, g : g + 1], axis=0
                    ),
                )

                ps = ppool.tile([P, D], F32)
                for h in range(0, D, 512):
                    nc.tensor.matmul(
                        ps[:, h : h + 512],
                        lhsT=ctall[:, g * P : (g + 1) * P],
                        rhs=up[:, h : h + 512],
                        start=True, stop=True,
                    )

                ot = opool.tile([P, D], F16)
                nc.scalar.activation(ot[:, 0:512], ps[:, 0:512], func=ACTF.Copy)
                nc.vector.tensor_copy(ot[:, 512:1024], ps[:, 512:1024])

                nc.sync.dma_start(out_d[g * P : (g + 1) * P, :], ot[:])

    nc.compile()
    return nc


def make_in_maps_v4(z: np.ndarray, p: np.ndarray, b: np.ndarray) -> list[dict]:
    TP = G3 * P
    maps = []
    for i in range(N_CORES):
        bi = b[i].astype(np.int64)
        idx = np.clip(np.cumsum(bi) - bi, 0, LZ - 1).astype(np.int64)
        idxp = np.concatenate([[idx[0]], idx])  # idxp[j] = idx[j-1], idxp[0] dummy
        gg = np.arange(G3)
        a = idxp[np.clip(127 * gg, 0, T)]       # a_g = idx[127g - 1]
        # gather offsets: contiguous span a_g + k (clipped)
        kk = np.arange(P)[:, None]
        idx_cm = np.clip(a[None, :] + kk, 0, LZ - 1).astype(np.int32)
        idx_cm = np.ascontiguousarray(idx_cm)
        # CT_all[k, 128g+m] = p'[m]*[k==l(m+1)] + (1-p'[m])*[k==l(m)]
        ct = np.zeros((P, TP), dtype=np.float32)
        pf = p[i].astype(np.float64)
        for g in range(G3):
            t0 = 127 * g
            mmax = min(127, T - t0)
            if mmax <= 0:
                continue
            mm = np.arange(mmax)
            tglob = t0 + mm
            pm = pf[tglob].copy()
            if g == 0:
                pm[0] = 1.0
            l_cur = idxp[tglob] - a[g]        # idx[t-1] - a_g
            l_next = idx[tglob] - a[g]        # idx[t]   - a_g
            if g == 0:
                l_cur[0] = 0
            cols = g * P + mm
            np.add.at(ct, (l_next, cols), pm)
            np.add.at(ct, (l_cur, cols), 1.0 - pm)
        m = {
            "z16": z[i].astype(np.float16),
            "ctall": ct.astype(np.float16),
            "idxc": idx_cm,
        }
        maps.append(m)
    return maps


def run_v4(z, p, b, **spmd_kwargs):
    key = "v4"
    if key not in _NC_CACHE:
        _NC_CACHE[key] = build_nc_v4()
    nc = _NC_CACHE[key]
    in_maps = make_in_maps_v4(z, p, b)
    res = run_bass_kernel_spmd(nc, in_maps, core_ids=list(range(N_CORES)), **spmd_kwargs)
    outs = []
    for i in range(N_CORES):
        st = res.results[i]["out"].reshape(G3, P, D)[:, : P - 1, :]
        outs.append(st.reshape(-1, D)[:T])
    return np.stack(outs, axis=0), res


# revision 20
# speedup vs baseline: 4.5819x; 1.0139x over previous
"""Trainium2 Bass kernel for nn_DechunkingLayer.

Full-input contract: kernel(z, p, b, original_len) with
  z [8, 1024, 1024] f32, p [8, 4096] f32, b [8, 4096] i32  ->  [8, 4096, 1024] f32

Sharding: data-parallel over batch — core i processes row i (cumsum / gather /
roll are independent per batch row).

v2 design (fp16 staging, fused shift+blend matmul):
  host:   idx = clip(cumsum(b)-b, 0, Lz-1)  (pure input marshalling)
          z16 = z.astype(f16); output returned as f16 -> f32 host-side.
          Halves device HBM traffic (16.5MB vs 33.7MB) and tunnel bytes.
  device, per 128-row t-tile g:
          up = z16[idx[t]]                 # gpsimd indirect gather, f16 rows
          ps = C_g @ up (+ halo)           # ONE PE matmul: C_g has p on the
                                           # diag and q=1-p on the subdiag, so
                                           # it does roll+blend in one pass;
                                           # the cross-tile halo row is a 2nd
                                           # tiny accumulate-matmul with
                                           # q[128g] at lhsT row 127.
          out[g] = ps (f16)                # ACT+DVE copy PSUM->SBUF, DMA out
  C_g^T is built on-chip: Pb = ones^T @ p_row (PE broadcast), then
  C^T = Ssub + (Id - Ssub) * Pb (2 DVE ops on [128,128] f16).
"""

import numpy as np

import concourse.bass as bass
import concourse.bacc as bacc
import concourse.tile as tile
from concourse import mybir
from concourse.bass_utils import run_bass_kernel_spmd

P = 128       # partitions / t-tile height
G = 32        # t-tiles = T // P
T = 4096
LZ = 1024
D = 1024
N_CORES = 8

F32 = mybir.dt.float32
F16 = mybir.dt.float16
I32 = mybir.dt.int32
ALU = mybir.AluOpType
ACTF = mybir.ActivationFunctionType

GCOLS = 1   # t-tiles gathered per indirect_dma_start (>1 is broken: device abort)
GATHER_MODE = "indirect"   # "indirect" | "dma_gather"
CH = 4      # t-tiles per dma_gather call


def _const_inputs_v2() -> dict[str, np.ndarray]:
    return {
        "ssub": np.eye(P, k=1, dtype=np.float16),   # lhsT[k,t]=1 iff k==t-1
        "dpm": (np.eye(P, dtype=np.float32)
                - np.eye(P, k=1, dtype=np.float32)).astype(np.float16),
        "ones1": np.ones((1, P), dtype=np.float16),
    }


def build_nc_v2(gcols: int | None = None, gather_mode: str | None = None) -> bacc.Bacc:
    if gcols is None:
        gcols = GCOLS
    if gather_mode is None:
        gather_mode = GATHER_MODE
    assert G % gcols == 0
    nc = bacc.Bacc("TRN2", target_bir_lowering=False, debug=False)

    z_d = nc.dram_tensor("z16", [LZ, D], F16, kind="ExternalInput")
    p_d = nc.dram_tensor("p16", [1, T], F16, kind="ExternalInput")
    idx_d = nc.dram_tensor("idxc", [P, G], I32, kind="ExternalInput")
    if gather_mode == "dma_gather":
        w16_d = nc.dram_tensor("w16i", [P, T // 16], mybir.dt.int16,
                               kind="ExternalInput")
    hoff_d = nc.dram_tensor("hoff", [G, 1], I32, kind="ExternalInput")
    hoffp_d = nc.dram_tensor("hoffp", [G, 1], I32, kind="ExternalInput")
    qh_d = nc.dram_tensor("qhc", [G, 1], F32, kind="ExternalInput")
    ph_d = nc.dram_tensor("phc", [G, 1], F32, kind="ExternalInput")
    ssub_d = nc.dram_tensor("ssub", [P, P], F16, kind="ExternalInput")
    dpm_d = nc.dram_tensor("dpm", [P, P], F16, kind="ExternalInput")
    ones1_d = nc.dram_tensor("ones1", [1, P], F16, kind="ExternalInput")
    out_d = nc.dram_tensor("out", [T, D], F16, kind="ExternalOutput")

    with tile.TileContext(nc) as tc:
        with (
            tc.tile_pool(name="consts", bufs=1) as cpool,
            tc.tile_pool(name="up", bufs=12) as upool,
            tc.tile_pool(name="ct", bufs=4) as ctpool,
            tc.tile_pool(name="cttmp", bufs=4) as tmppool,
            tc.tile_pool(name="outp", bufs=6) as opool,
            tc.tile_pool(name="pbps", bufs=1, space="PSUM") as pbpsum,
            tc.tile_pool(name="psum", bufs=3, space="PSUM") as ppool,
        ):
            # ---- constants / small inputs (idxc first: gathers need it) ----
            idxc = cpool.tile([P, G], I32)
            nc.sync.dma_start(idxc[:], idx_d[:, :])
            if gather_mode == "dma_gather":
                w16 = cpool.tile([P, T // 16], mybir.dt.int16)
                nc.sync.dma_start(w16[:], w16_d[:, :])
            ones1 = cpool.tile([1, P], F16)
            nc.sync.dma_start(ones1[:], ones1_d[:, :])
            p16 = cpool.tile([1, T], F16)
            nc.sync.dma_start(p16[:], p_d[:, :])
            ssub = cpool.tile([P, P], F16)
            nc.scalar.dma_start(ssub[:], ssub_d[:, :])
            dpm = cpool.tile([P, P], F16)
            nc.scalar.dma_start(dpm[:], dpm_d[:, :])
            hoff = cpool.tile([G, 1], I32)
            nc.scalar.dma_start(hoff[:], hoff_d[:, :])
            hoffp = cpool.tile([G, 1], I32)
            nc.scalar.dma_start(hoffp[:], hoffp_d[:, :])
            qh = cpool.tile([G, 1], F32)
            nc.scalar.dma_start(qh[:], qh_d[:, :])
            ph = cpool.tile([G, 1], F32)
            nc.scalar.dma_start(ph[:], ph_d[:, :])
            out_head_rows = out_d[:, :].rearrange("(g x) d -> g x d", x=P)[:, 0, :]

            # ---- broadcast p to all partitions once: pball[k, t] = p[t] ----
            pball = cpool.tile([P, T], F16)
            for j in range(T // 512):
                pb_ps = pbpsum.tile([P, 512], F32)
                nc.tensor.matmul(
                    pb_ps[:], lhsT=ones1[:], rhs=p16[0:1, j * 512 : (j + 1) * 512],
                    start=True, stop=True,
                )
                nc.scalar.activation(
                    pball[:, j * 512 : (j + 1) * 512], pb_ps[:], func=ACTF.Copy
                )

            # head rows: row0c[g] = p[128g]*z16[idx[128g]]
            #                      + q[128g]*z16[idx[128g-1]]
            # (partition-aligned on 32 partitions; patched into each ot tile's
            # row 0 by a small SBUF->SBUF DMA before the tile's out write)
            haloq = cpool.tile([G, D], F16)
            nc.gpsimd.indirect_dma_start(
                out=haloq[:],
                out_offset=None,
                in_=z_d[:, :],
                in_offset=bass.IndirectOffsetOnAxis(ap=hoff[:, 0:1], axis=0),
            )
            halop = cpool.tile([G, D], F16)
            nc.gpsimd.indirect_dma_start(
                out=halop[:],
                out_offset=None,
                in_=z_d[:, :],
                in_offset=bass.IndirectOffsetOnAxis(ap=hoffp[:, 0:1], axis=0),
            )
            h1 = cpool.tile([G, D], F16)
            nc.vector.tensor_scalar(
                h1[:], halop[:], ph[:, 0:1], None, op0=ALU.mult
            )
            row0c = cpool.tile([G, D], F16)
            nc.vector.scalar_tensor_tensor(
                row0c[:], in0=haloq[:], scalar=qh[:, 0:1], in1=h1[:],
                op0=ALU.mult, op1=ALU.add,
            )

            chunk = None
            for g in range(G):
                # -- gather up[t] = z16[idx[t]] --
                if gather_mode == "dma_gather":
                    j = g % CH
                    if j == 0:
                        c = g // CH
                        ncols = CH * P // 16
                        chunk = upool.tile([P, CH, D], F16)
                        nc.gpsimd.dma_gather(
                            out_ap=chunk[:],
                            in_ap=z_d[:, :],
                            idxs_ap=w16[:, c * ncols : (c + 1) * ncols],
                            num_idxs=CH * P,
                            num_idxs_reg=CH * P,
                            elem_size=D,
                        )
                    up = chunk[:, j, :]
                elif gcols > 1:
                    j = g % gcols
                    if j == 0:
                        c = g // gcols
                        chunk = upool.tile([P, gcols, D], F16)
                        nc.gpsimd.indirect_dma_start(
                            out=chunk[:],
                            out_offset=None,
                            in_=z_d[:, :],
                            in_offset=bass.IndirectOffsetOnAxis(
                                ap=idxc[:, c * gcols : (c + 1) * gcols], axis=0
                            ),
                        )
                    up = chunk[:, j, :]
                else:
                    up_t = upool.tile([P, D], F16)
                    up = up_t[:]
                    nc.gpsimd.indirect_dma_start(
                        out=up,
                        out_offset=None,
                        in_=z_d[:, :],
                        in_offset=bass.IndirectOffsetOnAxis(
                            ap=idxc[:, g : g + 1], axis=0
                        ),
                    )

                # -- build C_g^T = Ssub + (Id - Ssub) * broadcast(p_g) --
                tmp = tmppool.tile([P, P], F16)
                nc.vector.tensor_tensor(
                    tmp[:], dpm[:], pball[:, g * P : (g + 1) * P], ALU.mult
                )
                ct = ctpool.tile([P, P], F16)
                nc.vector.tensor_tensor(ct[:], tmp[:], ssub[:], ALU.add)

                # -- fused roll+blend matmul --
                ps = ppool.tile([P, D], F32)
                for h in range(0, D, 512):
                    nc.tensor.matmul(
                        ps[:, h : h + 512], lhsT=ct[:], rhs=up[:, h : h + 512],
                        start=True, stop=True,
                    )

                # -- PSUM -> SBUF f16, split across ACT and DVE --
                ot = opool.tile([P, D], F16)
                nc.scalar.activation(ot[:, 0:512], ps[:, 0:512], func=ACTF.Copy)
                nc.vector.tensor_copy(ot[:, 512:1024], ps[:, 512:1024])
                nc.sync.dma_start(out_d[g * P : (g + 1) * P, :], ot[:])


    nc.compile()
    return nc


_NC_CACHE: dict[str, bacc.Bacc] = {}


def get_nc_v2(gcols: int | None = None, gather_mode: str | None = None) -> bacc.Bacc:
    if gcols is None:
        gcols = GCOLS
    if gather_mode is None:
        gather_mode = GATHER_MODE
    key = f"v2:{gcols}:{gather_mode}"
    if key not in _NC_CACHE:
        _NC_CACHE[key] = build_nc_v2(gcols, gather_mode)
    return _NC_CACHE[key]


def make_in_maps_v2(z: np.ndarray, p: np.ndarray, b: np.ndarray) -> list[dict]:
    consts = _const_inputs_v2()
    maps = []
    for i in range(N_CORES):
        bi = b[i].astype(np.int64)
        idx = np.clip(np.cumsum(bi) - bi, 0, LZ - 1).astype(np.int32)
        idx_cm = np.ascontiguousarray(idx.reshape(G, P).T)  # [P, G]
        p16 = p[i].astype(np.float16).reshape(1, T).copy()
        # head rows: out[128g] = p[128g]*z16[idx[128g]] + q[128g]*z16[idx[128g-1]]
        # (for g=0: p=1, q=0 -> out[0] = z16[idx[0]])
        hoff = np.zeros((G, 1), dtype=np.int32)
        hoff[1:, 0] = idx[P - 1 :: P][: G - 1]
        hoffp = np.ascontiguousarray(idx[::P].reshape(G, 1))
        qh = np.zeros((G, 1), dtype=np.float32)
        qh[1:, 0] = (1.0 - p[i, P::P].astype(np.float64)).astype(np.float32)
        ph = p[i, ::P].astype(np.float32).reshape(G, 1).copy()
        ph[0, 0] = 1.0
        p16[0, 0] = 1.0
        m = {
            "z16": z[i].astype(np.float16),
            "p16": p16,
            "idxc": idx_cm,
            "hoff": hoff,
            "hoffp": hoffp,
            "qhc": qh,
            "phc": ph,
        }
        if GATHER_MODE == "dma_gather":
            # wrap-16 int16 layout: W[j, c] = idx[16c + j], replicated to 128
            w16 = np.ascontiguousarray(
                np.tile(idx.reshape(T // 16, 16).T.astype(np.int16), (P // 16, 1))
            )
            m["w16i"] = w16
        m.update(consts)
        maps.append(m)
    return maps


def run(z, p, b, **spmd_kwargs):
    nc = get_nc_v2()
    in_maps = make_in_maps_v2(z, p, b)
    res = run_bass_kernel_spmd(nc, in_maps, core_ids=list(range(N_CORES)), **spmd_kwargs)
    out = np.stack([res.results[i]["out"] for i in range(N_CORES)], axis=0)
    return out, res


KERNEL_V = "v4"   # "v2" | "v3" | "v4"


def kernel(z, p, b, original_len=None, **_ignored) -> np.ndarray:
    z = np.asarray(z)
    p = np.asarray(p)
    b = np.asarray(b)
    assert z.shape == (N_CORES, LZ, D), z.shape
    assert p.shape == (N_CORES, T), p.shape
    assert b.shape == (N_CORES, T), b.shape
    runner = {"v3": run_v3, "v4": run_v4}.get(KERNEL_V, run)
    out, _ = runner(z, p, b)
    return out.astype(np.float32, copy=False)

# ---------------------------------------------------------------------------
# v3: overlapping stride-127 tiles - the roll halo is INSIDE each tile's
# gather, so there are no cross-tile deps, no head-row fixups, and no DRAM
# WAW. Each tile g gathers up[k] = z16[idx[127g - 1 + k]] (k=0..127) and
# computes out[127g + m] = p*up[m+1] + q*up[m] for m=0..126 via one matmul
# with C'[k,m] = p'[m]@[k==m+1] + q'[m]@[k==m] = Id + (Slow - Id) * Pb.
# Tiles write disjoint aligned 128-row blocks of a [33*128, D] staging
# output; the host strips row 127 of each block.
# ---------------------------------------------------------------------------
G3 = 33  # ceil(T / 127)


def build_nc_v3() -> bacc.Bacc:
    nc = bacc.Bacc("TRN2", target_bir_lowering=False, debug=False)

    TP = G3 * P  # padded output rows / pov length
    z_d = nc.dram_tensor("z16", [LZ, D], F16, kind="ExternalInput")
    p_d = nc.dram_tensor("pov", [1, TP], F16, kind="ExternalInput")
    idx_d = nc.dram_tensor("idxc", [P, G3], I32, kind="ExternalInput")
    id_d = nc.dram_tensor("id128", [P, P], F16, kind="ExternalInput")
    dpm_d = nc.dram_tensor("dpm2", [P, P], F16, kind="ExternalInput")
    ones1_d = nc.dram_tensor("ones1", [1, P], F16, kind="ExternalInput")
    out_d = nc.dram_tensor("out", [TP, D], F16, kind="ExternalOutput")

    with tile.TileContext(nc) as tc:
        with (
            tc.tile_pool(name="consts", bufs=1) as cpool,
            tc.tile_pool(name="up", bufs=12) as upool,
            tc.tile_pool(name="ct", bufs=4) as ctpool,
            tc.tile_pool(name="cttmp", bufs=4) as tmppool,
            tc.tile_pool(name="outp", bufs=6) as opool,
            tc.tile_pool(name="pbps", bufs=1, space="PSUM") as pbpsum,
            tc.tile_pool(name="psum", bufs=3, space="PSUM") as ppool,
        ):
            idxc = cpool.tile([P, G3], I32)
            nc.sync.dma_start(idxc[:], idx_d[:, :])
            ones1 = cpool.tile([1, P], F16)
            nc.sync.dma_start(ones1[:], ones1_d[:, :])
            pov = cpool.tile([1, TP], F16)
            nc.sync.dma_start(pov[:], p_d[:, :])
            id128 = cpool.tile([P, P], F16)
            nc.scalar.dma_start(id128[:], id_d[:, :])
            dpm2 = cpool.tile([P, P], F16)
            nc.scalar.dma_start(dpm2[:], dpm_d[:, :])

            # broadcast pov to all partitions once: pball[k, t] = pov[t]
            pball = cpool.tile([P, TP], F16)
            for j in range(0, TP, 512):
                w = min(512, TP - j)
                pb_ps = pbpsum.tile([P, 512], F32)
                nc.tensor.matmul(
                    pb_ps[:, 0:w], lhsT=ones1[:], rhs=pov[0:1, j : j + w],
                    start=True, stop=True,
                )
                nc.scalar.activation(
                    pball[:, j : j + w], pb_ps[:, 0:w], func=ACTF.Copy
                )

            for g in range(G3):
                up = upool.tile([P, D], F16)
                nc.gpsimd.indirect_dma_start(
                    out=up[:],
                    out_offset=None,
                    in_=z_d[:, :],
                    in_offset=bass.IndirectOffsetOnAxis(
                        ap=idxc[:, g : g + 1], axis=0
                    ),
                )

                tmp = tmppool.tile([P, P], F16)
                nc.vector.tensor_tensor(
                    tmp[:], dpm2[:], pball[:, g * P : (g + 1) * P], ALU.mult
                )
                ct = ctpool.tile([P, P], F16)
                nc.vector.tensor_tensor(ct[:], tmp[:], id128[:], ALU.add)

                ps = ppool.tile([P, D], F32)
                for h in range(0, D, 512):
                    nc.tensor.matmul(
                        ps[:, h : h + 512], lhsT=ct[:], rhs=up[:, h : h + 512],
                        start=True, stop=True,
                    )

                ot = opool.tile([P, D], F16)
                nc.scalar.activation(ot[:, 0:512], ps[:, 0:512], func=ACTF.Copy)
                nc.vector.tensor_copy(ot[:, 512:1024], ps[:, 512:1024])

                nc.sync.dma_start(out_d[g * P : (g + 1) * P, :], ot[:])

    nc.compile()
    return nc


def _const_inputs_v3() -> dict[str, np.ndarray]:
    slow = np.eye(P, k=-1, dtype=np.float32)   # [k==m+1]
    iden = np.eye(P, dtype=np.float32)
    return {
        "id128": iden.astype(np.float16),
        "dpm2": (slow - iden).astype(np.float16),
        "ones1": np.ones((1, P), dtype=np.float16),
    }


def make_in_maps_v3(z: np.ndarray, p: np.ndarray, b: np.ndarray) -> list[dict]:
    consts = _const_inputs_v3()
    TP = G3 * P
    maps = []
    for i in range(N_CORES):
        bi = b[i].astype(np.int64)
        idx = np.clip(np.cumsum(bi) - bi, 0, LZ - 1).astype(np.int32)
        # idx_cm[k, g] = idx[127g - 1 + k]  (clipped; g=0,k=0 dummy killed by q=0)
        kk = np.arange(P)[:, None]
        gg = np.arange(G3)[None, :]
        src = np.clip(127 * gg - 1 + kk, 0, T - 1)
        idx_cm = np.ascontiguousarray(idx[src])
        # pov[128g + m] = p[127g + m] for m<127; col 127 of each block = 0
        pov = np.zeros(TP, dtype=np.float32)
        t_src = (127 * gg + np.arange(P)[:, None]).T  # [G3, P]
        valid = (np.arange(P)[None, :] < 127) & (t_src < T)
        pv = np.where(valid, p[i][np.clip(t_src, 0, T - 1)], 0.0)
        pov = pv.reshape(-1).astype(np.float32)
        pov[0] = 1.0  # out[0] = z[idx[0]] exactly
        m = {
            "z16": z[i].astype(np.float16),
            "pov": pov.astype(np.float16).reshape(1, TP),
            "idxc": idx_cm,
        }
        m.update(consts)
        maps.append(m)
    return maps


def run_v3(z, p, b, **spmd_kwargs):
    key = "v3"
    if key not in _NC_CACHE:
        _NC_CACHE[key] = build_nc_v3()
    nc = _NC_CACHE[key]
    in_maps = make_in_maps_v3(z, p, b)
    res = run_bass_kernel_spmd(nc, in_maps, core_ids=list(range(N_CORES)), **spmd_kwargs)
    outs = []
    for i in range(N_CORES):
        st = res.results[i]["out"].reshape(G3, P, D)[:, : P - 1, :]
        outs.append(st.reshape(-1, D)[:T])
    return np.stack(outs, axis=0), res

# ---------------------------------------------------------------------------
# v4: like v3 but the gather reads each tile's CONTIGUOUS span of z rows
# (offsets a_g + k, a_g = idx[127g-1]) so HBM reads are sequential and
# SDMA packets aggregate; the data-dependent expansion is folded into the
# blend matrix CT_all[k, 128g+m] = p'[m]*[k==l(m+1)] + q'[m]*[k==l(m)]
# (l = idx - a_g), which the host builds from p and idx and ships (1MB).
# ---------------------------------------------------------------------------


def build_nc_v4() -> bacc.Bacc:
    nc = bacc.Bacc("TRN2", target_bir_lowering=False, debug=False)

    TP = G3 * P
    z_d = nc.dram_tensor("z16", [LZ, D], F16, kind="ExternalInput")
    ct_d = nc.dram_tensor("ctall", [P, TP], F16, kind="ExternalInput")
    idx_d = nc.dram_tensor("idxc", [P, G3], I32, kind="ExternalInput")
    out_d = nc.dram_tensor("out", [TP, D], F16, kind="ExternalOutput")

    with tile.TileContext(nc) as tc:
        with (
            tc.tile_pool(name="consts", bufs=1) as cpool,
            tc.tile_pool(name="up", bufs=16) as upool,
            tc.tile_pool(name="outp", bufs=8) as opool,
            tc.tile_pool(name="psum", bufs=4, space="PSUM") as ppool,
        ):
            idxc = cpool.tile([P, G3], I32)
            nc.sync.dma_start(idxc[:], idx_d[:, :])
            ctall = cpool.tile([P, TP], F16)
            nc.scalar.dma_start(ctall[:], ct_d[:, :])

            for g in range(G3):
                up = upool.tile([P, D], F16)
                nc.gpsimd.indirect_dma_start(
                    out=up[:],
                    out_offset=None,
                    in_=z_d[:, :],
                    in_offset=bass.IndirectOffsetOnAxis(
                        ap=idxc[:, g : g + 1], axis=0
                    ),
                )

                ps = ppool.tile([P, D], F32)
                for h in range(0, D, 512):
                    nc.tensor.matmul(
                        ps[:, h : h + 512],
                        lhsT=ctall[:, g * P : (g + 1) * P],
                        rhs=up[:, h : h + 512],
                        start=True, stop=True,
                    )

                ot = opool.tile([P, D], F16)
                nc.scalar.activation(ot[:, 0:512], ps[:, 0:512], func=ACTF.Copy)
                nc.vector.tensor_copy(ot[:, 512:1024], ps[:, 512:1024])

                nc.sync.dma_start(out_d[g * P : (g + 1) * P, :], ot[:])

    nc.compile()
    return nc


def make_in_maps_v4(z: np.ndarray, p: np.ndarray, b: np.ndarray) -> list[dict]:
    TP = G3 * P
    maps = []
    for i in range(N_CORES):
        bi = b[i].astype(np.int64)
        idx = np.clip(np.cumsum(bi) - bi, 0, LZ - 1).astype(np.int64)
        idxp = np.concatenate([[idx[0]], idx])  # idxp[j] = idx[j-1], idxp[0] dummy
        gg = np.arange(G3)
        a = idxp[np.clip(127 * gg, 0, T)]       # a_g = idx[127g - 1]
        # gather offsets: contiguous span a_g + k (clipped)
        kk = np.arange(P)[:, None]
        idx_cm = np.clip(a[None, :] + kk, 0, LZ - 1).astype(np.int32)
        idx_cm = np.ascontiguousarray(idx_cm)
        # CT_all[k, 128g+m] = p'[m]*[k==l(m+1)] + (1-p'[m])*[k==l(m)]
        ct = np.zeros((P, TP), dtype=np.float32)
        pf = p[i].astype(np.float64)
        for g in range(G3):
            t0 = 127 * g
            mmax = min(127, T - t0)
            if mmax <= 0:
                continue
            mm = np.arange(mmax)
            tglob = t0 + mm
            pm = pf[tglob].copy()
            if g == 0:
                pm[0] = 1.0
            l_cur = idxp[tglob] - a[g]        # idx[t-1] - a_g
            l_next = idx[tglob] - a[g]        # idx[t]   - a_g
            if g == 0:
                l_cur[0] = 0
            cols = g * P + mm
            np.add.at(ct, (l_next, cols), pm)
            np.add.at(ct, (l_cur, cols), 1.0 - pm)
        m = {
            "z16": z[i].astype(np.float16),
            "ctall": ct.astype(np.float16),
            "idxc": idx_cm,
        }
        maps.append(m)
    return maps


def run_v4(z, p, b, **spmd_kwargs):
    key = "v4"
    if key not in _NC_CACHE:
        _NC_CACHE[key] = build_nc_v4()
    nc = _NC_CACHE[key]
    in_maps = make_in_maps_v4(z, p, b)
    res = run_bass_kernel_spmd(nc, in_maps, core_ids=list(range(N_CORES)), **spmd_kwargs)
    outs = []
    for i in range(N_CORES):
        st = res.results[i]["out"].reshape(G3, P, D)[:, : P - 1, :]
        outs.append(st.reshape(-1, D)[:T])
    return np.stack(outs, axis=0), res
